# revision 27
# baseline (speedup 1.0000x reference)
"""Causal multi-head attention on 8 Trainium2 NeuronCores.

Problem: x[B=4,S=2048,E=1024], Wq/Wk/Wv[H=16,E,D=64], Wo[E,E], bo[E].
  out = softmax_causal(q k^T / sqrt(D)) v, heads concat, @ Wo.T + bo

Sharding (tensor parallel over heads, data parallel over batch):
  core c -> (batch b = c//2, head-group g = c%2 of 8 heads).
  Each core: QKV projections + attention for its 8 heads of its batch,
  normalization, and a FULL-width output-projection partial from its own
  heads (Wo split on its input axis); a pairwise ReduceScatter(add) over
  (2b, 2b+1) sums the partials and leaves each core its 512 output
  columns, written straight into the kernel output.  No peer data is ever
  needed by compute -> no collective on the critical path except the
  final add.

Kernel internals (per core), v3:
  - All SBUF data bf16 (psum f32).  Activations transposed: xT[E,S],
    QT/KT[dg,S], scoresT[t,q]; softmax denominator from a ones-column
    appended to V; probabilities feed the AV matmul moving operand.
  - Scores for a head PAIR (row-tiled 64-contraction matmuls at PE row
    groups 0/64) share one [128, 1024] psum tile; ONE exp activation
    covers both heads.
  - Causality at 128 granularity: suffix-restricted scores/exp/AV on
    diagonal tiles + a [128,128] upper-tri wedge multiply.
  - 1/denom via reciprocal_approx_fast on a per-chunk [8,512] stack
    (gathered by tiny SBUF->SBUF DMAs); broadcast across each head's 64
    rows by a small selector matmul; bf16 in-place normalize.
  - Phase-1 (QKV) and out-projection work interleaved into the attention
    emission so TensorE always has dense work while ScalarE exp runs
    (HAM stays at 2.4 GHz).  x is loaded in [128,512] column slices so
    the prologue is never DMA-starved.
"""

import os
import sys

for _p in ("/opt/trn_rl_repo", "/root/.axon_site/_ro/trn_rl_repo"):
    if os.path.isdir(_p) and _p not in sys.path:
        sys.path.append(_p)

import numpy as np
import ml_dtypes

import concourse.bass as bass
import concourse.mybir as mybir
import concourse.tile as tile
from concourse import bacc

B, S, E, H, D = 4, 2048, 1024, 16, 64
NCORES = 8
G = 2  # head groups
HL = H // G  # heads per core = 8
DG = HL * D  # local head dim = 512
EH = E // G  # final output columns per core = 512
P = 128
SC = 512  # sequence chunk
NSC = S // SC  # 4
NT = S // P  # 16 key tiles
ET = E // P  # 8 embedding tiles
ND = DG // P  # 4 head-pairs per core
SCALE = 1.0 / np.sqrt(D)

F32 = mybir.dt.float32
BF16 = mybir.dt.bfloat16

_CACHE = {}


def _build_nc():
    nc = bacc.Bacc("TRN2", target_bir_lowering=False, debug=False, num_devices=NCORES)

    xT = nc.dram_tensor("xT", [E, S], BF16, kind="ExternalInput")
    wq = nc.dram_tensor("wq", [E, DG], BF16, kind="ExternalInput")
    wk = nc.dram_tensor("wk", [E, DG], BF16, kind="ExternalInput")
    wv = nc.dram_tensor("wv", [E, DG], BF16, kind="ExternalInput")
    wo2 = nc.dram_tensor("wo2", [DG, E], BF16, kind="ExternalInput")
    bo2 = nc.dram_tensor("bo2", [P, ET], F32, kind="ExternalInput")
    mask = nc.dram_tensor("mask", [P, P], BF16, kind="ExternalInput")
    sel8 = nc.dram_tensor("sel8", [HL, ND * P], BF16, kind="ExternalInput")
    outT = nc.dram_tensor("outT", [NSC, EH, SC], BF16, kind="ExternalOutput")

    with tile.TileContext(nc) as tc:
        with (
            tc.tile_pool(name="persist", bufs=1) as persist,
            tc.tile_pool(name="expp", bufs=4) as expp,
            tc.tile_pool(name="attnp", bufs=8) as attnp,
            tc.tile_pool(name="dnp", bufs=4) as dnp,
            tc.tile_pool(name="dinvp", bufs=2) as dinvp,
            tc.tile_pool(name="workp", bufs=4) as workp,
            tc.tile_pool(name="outp", bufs=3) as outp,
            tc.tile_pool(name="psc", bufs=2, space="PSUM") as psc,
            tc.tile_pool(name="patt", bufs=2, space="PSUM") as patt,
            tc.tile_pool(name="pmisc", bufs=2, space="PSUM") as pmisc,
            tc.tile_pool(name="dram", bufs=1, space="DRAM") as dram,
        ):
            # ---- persistent tiles ----
            xs = [
                [persist.tile([P, SC], BF16, name=f"x{e}_{c}") for c in range(NSC)]
                for e in range(ET)
            ]
            wq_sb = persist.tile([P, ET, DG], BF16, name="wq")
            wk_sb = persist.tile([P, ET, DG], BF16, name="wk")
            wv_sb = persist.tile([P, ET, DG], BF16, name="wv")
            wo_sb = persist.tile([P, ND, E], BF16, name="wo")
            bo_sb = persist.tile([P, ET], F32, name="bo")
            mask_sb = persist.tile([P, P], BF16, name="mask")
            sel_sb = persist.tile([HL, ND * P], BF16, name="sel")
            kt = [
                [persist.tile([P, SC], BF16, name=f"kt{d}_{kc}") for kc in range(NSC)]
                for d in range(ND)
            ]
            qt = [
                [persist.tile([P, SC], BF16, name=f"qt{d}_{sc}") for sc in range(NSC)]
                for d in range(ND)
            ]
            v_sb = [persist.tile([P, HL, D + 1], BF16, name=f"v{t}") for t in range(NT)]

            cc_in = dram.tile([NSC, E, SC], BF16)
            cc_rs = dram.tile([NSC, 2, EH // 2, SC], BF16)

            # ---- input DMAs: per-e weight blocks interleaved with x slices
            # so K(0)'s e-th matmul can start as soon as its operands land
            for e in range(ET):
                nc.sync.dma_start(
                    wk_sb[:, e, :], wk[P * e : P * (e + 1), :]
                )
                nc.sync.dma_start(xs[e][0][:], xT[P * e : P * (e + 1), 0:SC])
            for e in range(ET):
                nc.sync.dma_start(wq_sb[:, e, :], wq[P * e : P * (e + 1), :])
            for e in range(ET):
                nc.sync.dma_start(wv_sb[:, e, :], wv[P * e : P * (e + 1), :])
            for c in range(1, NSC):
                for e in range(ET):
                    nc.sync.dma_start(
                        xs[e][c][:], xT[P * e : P * (e + 1), SC * c : SC * (c + 1)]
                    )
            nc.sync.dma_start(wo_sb[:], wo2.rearrange("(ko p) m -> p ko m", p=P))
            nc.sync.dma_start(bo_sb[:], bo2[:])
            nc.sync.dma_start(mask_sb[:], mask[:])
            nc.sync.dma_start(sel_sb[:], sel8[:])
            for t in range(NT):
                nc.vector.memset(v_sb[t][:, :, D], 1.0)

            # ---- phase-1 emitters ----
            def emit_k(d, kc, w_sb=wk_sb, dst=kt):
                acc = pmisc.tile([P, SC], F32, tag="m", name="acc")
                for e in range(ET):
                    nc.tensor.matmul(
                        acc[:],
                        w_sb[:, e, P * d : P * (d + 1)],
                        xs[e][kc][:],
                        start=(e == 0),
                        stop=(e == ET - 1),
                    )
                nc.vector.tensor_copy(dst[d][kc][:], acc[:])

            def emit_q(d, sc):
                emit_k(d, sc, w_sb=wq_sb, dst=qt)

            def emit_v(t):
                acc = pmisc.tile([P, DG], F32, tag="m", name="accv")
                for e in range(ET):
                    nc.tensor.matmul(
                        acc[:],
                        xs[e][t // 4][:, P * (t % 4) : P * (t % 4 + 1)],
                        wv_sb[:, e, :],
                        start=(e == 0),
                        stop=(e == ET - 1),
                    )
                nc.vector.tensor_copy(
                    v_sb[t][:, :, 0:D], acc[:].rearrange("p (h d) -> p h d", d=D)
                )

            # ---- normalization (per-d, local) ----
            # dn_loc rows beyond the d-th pair still hold the 1.0 memset;
            # their reciprocals are finite and zero-weighted by sel.
            def op_norm(d, attn_t, dn_loc):
                dn_f = dinvp.tile([HL, SC], F32, tag="dnf", name="dn_f")
                dn_i = dinvp.tile([HL, SC], F32, tag="dni", name="dn_i")
                dinv_b = dinvp.tile([HL, SC], BF16, tag="dnb", name="dinv_b")
                nc.vector.tensor_copy(dn_f[:], dn_loc[:])
                nc.vector.reciprocal_approx_fast(dn_i[:], dn_f[:])
                nc.vector.tensor_copy(dinv_b[:], dn_i[:])
                bc = pmisc.tile([P, SC], F32, tag="m", name="bc")
                nc.tensor.matmul(
                    bc[:], sel_sb[:, P * d : P * (d + 1)], dinv_b[:],
                    start=True, stop=True,
                )
                bc_sb = workp.tile([P, SC], BF16, tag="bc")
                nc.vector.tensor_copy(bc_sb[:], bc[:])
                nc.vector.tensor_mul(attn_t[:], attn_t[:], bc_sb[:])

            # cc_in row order interleaves the two ranks' e-quarters so each
            # half [0:512] / [512:1024] ReduceScatters to the right slices:
            # rows = [e 0:256 | e 512:768 | e 256:512 | e 768:1024]
            CPERM = {0: 0, 1: 1, 2: 4, 3: 5, 4: 2, 5: 3, 6: 6, 7: 7}

            def op_proj(sc, et, at):
                acc = pmisc.tile([P, SC], F32, tag="m", name="acco")
                for d in range(ND):
                    nc.tensor.matmul(
                        acc[:],
                        wo_sb[:, d, P * et : P * (et + 1)],
                        at[d][:],
                        start=(d == 0),
                        stop=(d == ND - 1),
                    )
                stage = outp.tile([P, SC], BF16, tag="ot")
                nc.vector.tensor_scalar_add(stage[:], acc[:], bo_sb[:, et : et + 1])
                r = P * CPERM[et]
                nc.sync.dma_start(cc_in[sc, r : r + P, :], stage[:])

            def op_rs(sc, h):
                nc.gpsimd.collective_compute(
                    "ReduceScatter",
                    mybir.AluOpType.add,
                    replica_groups=[[0, 1], [2, 3], [4, 5], [6, 7]],
                    ins=[cc_in[sc, (E // 2) * h : (E // 2) * (h + 1), :].opt()],
                    outs=[cc_rs[sc, h].opt()],
                )
                nc.sync.dma_start(
                    outT[sc, (EH // 2) * h : (EH // 2) * (h + 1), :], cc_rs[sc, h]
                )

            def out_projection_fills(sc, at):
                fills = []
                for h in range(2):
                    for et in ([0, 1, 4, 5] if h == 0 else [2, 3, 6, 7]):
                        fills.append(lambda et=et: op_proj(sc, et, at))
                    fills.append(lambda h=h: op_rs(sc, h))
                return fills

            # ---- attention ----
            def attention_chunk(sc, fills):
                nt = 4 * (sc + 1)
                n_iter = ND * nt
                it = 0
                popped = 0
                at = []  # per-d AV tiles [128, 512] (bf16), normalized in place
                dn_loc = dnp.tile([HL, SC], BF16, tag="dnl", name="dn_loc")
                nc.vector.memset(dn_loc[:], 1.0)
                for d in range(ND):
                    att = [
                        patt.tile([D + 1, SC], F32, tag="att", name=f"att{h}")
                        for h in range(2)
                    ]

                    def emit_av(j, o, ex, d=d, att=att, nt=nt):
                        for half in range(2):
                            nc.tensor.matmul(
                                att[half][:, o:SC],
                                v_sb[j][:, 2 * d + half, :],
                                ex[:, SC * half + o : SC * (half + 1)],
                                start=(j == 0),
                                stop=(j == nt - 1),
                            )

                    prev = None
                    for j in range(nt):
                        m = j - 4 * sc
                        o = P * m if m > 0 else 0
                        sco = psc.tile([P, 2 * SC], F32, tag="sc", name="sco")
                        for half in range(2):
                            r = D * half
                            nc.tensor.matmul(
                                sco[:, SC * half + o : SC * (half + 1)],
                                kt[d][j // 4][r : r + D, P * (j % 4) : P * (j % 4 + 1)],
                                qt[d][sc][r : r + D, o:SC],
                                start=True,
                                stop=True,
                                tile_position=(r, 0),
                            )
                        ex = expp.tile([P, 2 * SC], BF16, tag="ex", name="ex")
                        if o > 0:
                            for half in range(2):
                                nc.scalar.activation(
                                    ex[:, SC * half + o : SC * (half + 1)],
                                    sco[:, SC * half + o : SC * (half + 1)],
                                    mybir.ActivationFunctionType.Exp,
                                    scale=SCALE,
                                )
                        else:
                            nc.scalar.activation(
                                ex[:],
                                sco[:],
                                mybir.ActivationFunctionType.Exp,
                                scale=SCALE,
                            )
                        if m >= 0:  # diagonal tile: mask the wedge
                            for half in range(2):
                                c0 = SC * half + P * m
                                nc.vector.tensor_mul(
                                    ex[:, c0 : c0 + P], ex[:, c0 : c0 + P], mask_sb[:]
                                )
                        if prev is not None:
                            emit_av(*prev)
                        prev = (j, o, ex)
                        it += 1
                        want = (it * len(fills)) // n_iter
                        while popped < want:
                            fills[popped]()
                            popped += 1
                    emit_av(*prev)
                    # stage raw AV + denominators, release att psum
                    attn_t = attnp.tile([P, SC], BF16, tag="at", name="attn_t")
                    for half in range(2):
                        nc.scalar.copy(
                            attn_t[D * half : D * (half + 1), :], att[half][0:D, :]
                        )
                        dnrow = dnp.tile([1, SC], BF16, tag="dn", name="dnrow")
                        nc.vector.tensor_copy(dnrow[:], att[half][D : D + 1, :])
                        nc.sync.dma_start(
                            dn_loc[2 * d + half : 2 * d + half + 1, :], dnrow[:]
                        )
                    op_norm(d, attn_t, dn_loc)
                    at.append(attn_t)
                while popped < len(fills):
                    fills[popped]()
                    popped += 1
                return at

            # ---- schedule ----
            for d in range(ND):
                emit_k(d, 0)
            for d in range(ND):
                emit_q(d, 0)
            for t in range(4):
                emit_v(t)

            prev_chunk = None
            for sc in range(NSC):
                fills = []
                if sc + 1 < NSC:
                    for d in range(ND):
                        fills.append(lambda d=d, kc=sc + 1: emit_k(d, kc))
                    for t in range(4 * (sc + 1), 4 * (sc + 2)):
                        fills.append(lambda t=t: emit_v(t))
                    for d in range(ND):
                        fills.append(lambda d=d, qc=sc + 1: emit_q(d, qc))
                if prev_chunk is not None:
                    fills.extend(out_projection_fills(sc - 1, prev_chunk))
                prev_chunk = attention_chunk(sc, fills)
            for f in out_projection_fills(NSC - 1, prev_chunk):
                f()

    nc.compile()
    return nc


def _get_runner():
    """Build (once) and return a callable in_maps -> list of out_maps."""
    if "runner" in _CACHE:
        return _CACHE["runner"]

    nc = _build_nc()
    _CACHE["nc"] = nc

    import jax
    from jax.sharding import Mesh, PartitionSpec
    from jax.experimental.shard_map import shard_map
    from concourse import bass2jax
    from concourse.bass2jax import _bass_exec_p, partition_id_tensor

    bass2jax.install_neuronx_cc_hook()

    in_names, out_names, out_avals, zero_shapes = [], [], [], []
    partition_name = nc.partition_id_tensor.name if nc.partition_id_tensor else None
    for alloc in nc.m.functions[0].allocations:
        if not isinstance(alloc, mybir.MemoryLocationSet):
            continue
        name = alloc.memorylocations[0].name
        if alloc.kind == "ExternalInput":
            if name != partition_name:
                in_names.append(name)
        elif alloc.kind == "ExternalOutput":
            out_names.append(name)
            shape = tuple(alloc.tensor_shape)
            dtype = mybir.dt.np(alloc.dtype)
            out_avals.append(jax.core.ShapedArray(shape, dtype))
            zero_shapes.append((shape, dtype))
    n_params = len(in_names)
    all_in_names = list(in_names) + list(out_names)
    if partition_name is not None:
        all_in_names.append(partition_name)

    def _body(*args):
        operands = list(args)
        if partition_name is not None:
            operands.append(partition_id_tensor())
        outs = _bass_exec_p.bind(
            *operands,
            out_avals=tuple(out_avals),
            in_names=tuple(all_in_names),
            out_names=tuple(out_names),
            lowering_input_output_aliases=(),
            sim_require_finite=True,
            sim_require_nnan=True,
            nc=nc,
        )
        return tuple(outs)

    devices = jax.devices()[:NCORES]
    mesh = Mesh(np.asarray(devices), ("core",))
    n_outs = len(out_names)
    sharded = jax.jit(
        shard_map(
            _body,
            mesh=mesh,
            in_specs=(PartitionSpec("core"),) * (n_params + n_outs),
            out_specs=(PartitionSpec("core"),) * n_outs,
            check_rep=False,
        ),
        donate_argnums=tuple(range(n_params, n_params + n_outs)),
        keep_unused=True,
    )

    def runner(in_maps):
        per_core = [[np.asarray(m[name]) for name in in_names] for m in in_maps]
        concat_in = [
            np.concatenate([per_core[c][i] for c in range(NCORES)], axis=0)
            for i in range(n_params)
        ]
        concat_zeros = [
            np.zeros((NCORES * s[0], *s[1:]), d) for (s, d) in zero_shapes
        ]
        out_arrs = sharded(*concat_in, *concat_zeros)
        return [
            {
                name: np.asarray(out_arrs[i]).reshape(NCORES, *out_avals[i].shape)[c]
                for i, name in enumerate(out_names)
            }
            for c in range(NCORES)
        ]

    _CACHE["runner"] = runner
    _CACHE["sharded"] = sharded
    _CACHE["mesh"] = mesh
    _CACHE["meta"] = (in_names, out_names, zero_shapes)
    return runner


def timing_setup(in_maps):
    """Device-resident timing: returns (make_zeros, call).

    `call(make_zeros())` runs one on-device execution with inputs already
    resident (zeros are donated output buffers, created outside the timer).
    """
    _get_runner()
    import jax
    from jax.sharding import NamedSharding, PartitionSpec

    in_names, out_names, zero_shapes = _CACHE["meta"]
    sharding = NamedSharding(_CACHE["mesh"], PartitionSpec("core"))
    per_core = [[np.asarray(m[name]) for name in in_names] for m in in_maps]
    dev_in = [
        jax.device_put(
            np.concatenate([per_core[c][i] for c in range(NCORES)], axis=0), sharding
        )
        for i in range(len(in_names))
    ]
    jax.block_until_ready(dev_in)

    def make_zeros():
        zs = [
            jax.device_put(np.zeros((NCORES * s[0], *s[1:]), d), sharding)
            for (s, d) in zero_shapes
        ]
        jax.block_until_ready(zs)
        return zs

    def call(zs):
        out = _CACHE["sharded"](*dev_in, *zs)
        jax.block_until_ready(out)
        return out

    return make_zeros, call


def make_in_maps(x, Wq, Wk, Wv, Wo, bo):
    """Host-side sharding: slice/transpose/cast full inputs into per-core maps."""
    x = np.asarray(x, dtype=np.float32)
    Wq = np.asarray(Wq, dtype=np.float32)
    Wk = np.asarray(Wk, dtype=np.float32)
    Wv = np.asarray(Wv, dtype=np.float32)
    Wo = np.asarray(Wo, dtype=np.float32)
    bo = np.asarray(bo, dtype=np.float32)
    bf = ml_dtypes.bfloat16

    mask = np.triu(np.ones((P, P), dtype=bf))  # keep t <= s
    sel = np.zeros((HL, ND * P), dtype=bf)
    for d in range(ND):
        sel[2 * d, P * d : P * d + D] = 1
        sel[2 * d + 1, P * d + D : P * (d + 1)] = 1
    bo2 = np.ascontiguousarray((0.5 * bo).reshape(ET, P).T)  # [P, ET]
    WoT = np.ascontiguousarray(Wo.T)  # [dg_full, e]
    in_maps = []
    for c in range(NCORES):
        b, g = c // 2, c % 2
        xT = np.ascontiguousarray(x[b].T).astype(bf)  # [E, S]
        wq = np.ascontiguousarray(
            Wq[HL * g : HL * (g + 1)].transpose(1, 0, 2).reshape(E, DG)
        ).astype(bf)
        wk = np.ascontiguousarray(
            Wk[HL * g : HL * (g + 1)].transpose(1, 0, 2).reshape(E, DG)
        ).astype(bf)
        wv = np.ascontiguousarray(
            Wv[HL * g : HL * (g + 1)].transpose(1, 0, 2).reshape(E, DG)
        ).astype(bf)
        wo2 = np.ascontiguousarray(WoT[DG * g : DG * (g + 1), :]).astype(bf)
        in_maps.append(
            {
                "xT": xT,
                "wq": wq,
                "wk": wk,
                "wv": wv,
                "wo2": wo2,
                "bo2": bo2,
                "mask": mask,
                "sel8": sel,
            }
        )
    return in_maps


def assemble_output(results):
    """Gather per-core outT [EH, S] slices into the full [B, S, E] output."""
    out = np.empty((B, S, E), dtype=np.float32)
    for c in range(NCORES):
        b, g = c // 2, c % 2
        o = results[c]["outT"]  # [NSC, EH, SC]
        out[b, :, EH * g : EH * (g + 1)] = (
            o.transpose(0, 2, 1).reshape(S, EH).astype(np.float32)
        )
    return out


def kernel(x, Wq, Wk, Wv, Wo, bo):
    runner = _get_runner()
    in_maps = make_in_maps(x, Wq, Wk, Wv, Wo, bo)
    results = runner(in_maps)
    return assemble_output(results)


# revision 34
# speedup vs baseline: 1.1206x; 1.1206x over previous
"""Causal multi-head attention on 8 Trainium2 NeuronCores.

Problem: x[B=4,S=2048,E=1024], Wq/Wk/Wv[H=16,E,D=64], Wo[E,E], bo[E].
  out = softmax_causal(q k^T / sqrt(D)) v, heads concat, @ Wo.T + bo

Sharding (tensor parallel over heads, data parallel over batch):
  core c -> (batch b = c//2, head-group g = c%2 of 8 heads).
  Each core: QKV projections + attention for its 8 heads of its batch,
  normalization, and a FULL-width output-projection partial from its own
  heads (Wo split on its input axis); a pairwise ReduceScatter(add) over
  (2b, 2b+1) sums the partials and leaves each core its 512 output
  columns, written straight into the kernel output.  No peer data is ever
  needed by compute -> no collective on the critical path except the
  final add.

Kernel internals (per core), v3:
  - All SBUF data bf16 (psum f32).  Activations transposed: xT[E,S],
    QT/KT[dg,S], scoresT[t,q]; softmax denominator from a ones-column
    appended to V; probabilities feed the AV matmul moving operand.
  - Scores for a head PAIR (row-tiled 64-contraction matmuls at PE row
    groups 0/64) share one [128, 1024] psum tile; ONE exp activation
    covers both heads.
  - Causality at 128 granularity: suffix-restricted scores/exp/AV on
    diagonal tiles + a [128,128] upper-tri wedge multiply.
  - 1/denom via reciprocal_approx_fast on a per-chunk [8,512] stack
    (gathered by tiny SBUF->SBUF DMAs); broadcast across each head's 64
    rows by a small selector matmul; bf16 in-place normalize.
  - Phase-1 (QKV) and out-projection work interleaved into the attention
    emission so TensorE always has dense work while ScalarE exp runs
    (HAM stays at 2.4 GHz).  x is loaded in [128,512] column slices so
    the prologue is never DMA-starved.
"""

import os
import sys

for _p in ("/opt/trn_rl_repo", "/root/.axon_site/_ro/trn_rl_repo"):
    if os.path.isdir(_p) and _p not in sys.path:
        sys.path.append(_p)

import numpy as np
import ml_dtypes

import concourse.bass as bass
import concourse.mybir as mybir
import concourse.tile as tile
from concourse import bacc

B, S, E, H, D = 4, 2048, 1024, 16, 64
NCORES = 8
G = 2  # head groups
HL = H // G  # heads per core = 8
DG = HL * D  # local head dim = 512
EH = E // G  # final output columns per core = 512
P = 128
SC = 512  # sequence chunk
NSC = S // SC  # 4
NT = S // P  # 16 key tiles
ET = E // P  # 8 embedding tiles
ND = DG // P  # 4 head-pairs per core
SCALE = 1.0 / np.sqrt(D)

F32 = mybir.dt.float32
BF16 = mybir.dt.bfloat16

_CACHE = {}


def _build_nc():
    nc = bacc.Bacc("TRN2", target_bir_lowering=False, debug=False, num_devices=NCORES)

    xT = nc.dram_tensor("xT", [E, S], BF16, kind="ExternalInput")
    wq = nc.dram_tensor("wq", [E, DG], BF16, kind="ExternalInput")
    wk = nc.dram_tensor("wk", [E, DG], BF16, kind="ExternalInput")
    wv = nc.dram_tensor("wv", [E, DG], BF16, kind="ExternalInput")
    wo2 = nc.dram_tensor("wo2", [DG, E], BF16, kind="ExternalInput")
    bo2 = nc.dram_tensor("bo2", [P, ET], F32, kind="ExternalInput")
    mask = nc.dram_tensor("mask", [P, P], BF16, kind="ExternalInput")
    sel8 = nc.dram_tensor("sel8", [HL, ND * P], BF16, kind="ExternalInput")
    outT = nc.dram_tensor("outT", [NSC, EH, SC], BF16, kind="ExternalOutput")

    with tile.TileContext(nc) as tc:
        with (
            tc.tile_pool(name="persist", bufs=1) as persist,
            tc.tile_pool(name="expp", bufs=4) as expp,
            tc.tile_pool(name="attnp", bufs=8) as attnp,
            tc.tile_pool(name="dnp", bufs=4) as dnp,
            tc.tile_pool(name="dinvp", bufs=2) as dinvp,
            tc.tile_pool(name="workp", bufs=4) as workp,
            tc.tile_pool(name="outp", bufs=3) as outp,
            tc.tile_pool(name="psc", bufs=2, space="PSUM") as psc,
            tc.tile_pool(name="patt", bufs=2, space="PSUM") as patt,
            tc.tile_pool(name="pmisc", bufs=2, space="PSUM") as pmisc,
            tc.tile_pool(name="dram", bufs=1, space="DRAM") as dram,
        ):
            # ---- persistent tiles ----
            xs = [
                [persist.tile([P, SC], BF16, name=f"x{e}_{c}") for c in range(NSC)]
                for e in range(ET)
            ]
            wq_sb = persist.tile([P, ET, DG], BF16, name="wq")
            wk_sb = persist.tile([P, ET, DG], BF16, name="wk")
            wv_sb = persist.tile([P, ET, DG], BF16, name="wv")
            wo_sb = persist.tile([P, ND, E], BF16, name="wo")
            bo_sb = persist.tile([P, ET], F32, name="bo")
            mask_sb = persist.tile([P, P], BF16, name="mask")
            sel_sb = persist.tile([HL, ND * P], BF16, name="sel")
            kt = [
                [persist.tile([P, SC], BF16, name=f"kt{d}_{kc}") for kc in range(NSC)]
                for d in range(ND)
            ]
            qt = [
                [persist.tile([P, SC], BF16, name=f"qt{d}_{sc}") for sc in range(NSC)]
                for d in range(ND)
            ]
            v_sb = [persist.tile([P, HL, D + 1], BF16, name=f"v{t}") for t in range(NT)]

            cc_in = dram.tile([NSC, E, SC], BF16)
            cc_rs = dram.tile([NSC, 2, EH // 2, SC], BF16)

            # ---- input DMAs: per-e weight blocks interleaved with x slices
            # so K(0)'s e-th matmul can start as soon as its operands land
            for e in range(ET):
                nc.sync.dma_start(
                    wk_sb[:, e, :], wk[P * e : P * (e + 1), :]
                )
                nc.sync.dma_start(xs[e][0][:], xT[P * e : P * (e + 1), 0:SC])
            for e in range(ET):
                nc.sync.dma_start(wq_sb[:, e, :], wq[P * e : P * (e + 1), :])
            for e in range(ET):
                nc.sync.dma_start(wv_sb[:, e, :], wv[P * e : P * (e + 1), :])
            for c in range(1, NSC):
                for e in range(ET):
                    nc.sync.dma_start(
                        xs[e][c][:], xT[P * e : P * (e + 1), SC * c : SC * (c + 1)]
                    )
            nc.sync.dma_start(wo_sb[:], wo2.rearrange("(ko p) m -> p ko m", p=P))
            nc.sync.dma_start(bo_sb[:], bo2[:])
            nc.sync.dma_start(mask_sb[:], mask[:])
            nc.sync.dma_start(sel_sb[:], sel8[:])
            for t in range(NT):
                nc.vector.memset(v_sb[t][:, :, D], 1.0)

            # ---- phase-1 emitters ----
            def emit_k(d, kc, w_sb=wk_sb, dst=kt):
                acc = pmisc.tile([P, SC], F32, tag="m", name="acc")
                for e in range(ET):
                    nc.tensor.matmul(
                        acc[:],
                        w_sb[:, e, P * d : P * (d + 1)],
                        xs[e][kc][:],
                        start=(e == 0),
                        stop=(e == ET - 1),
                    )
                nc.vector.tensor_copy(dst[d][kc][:], acc[:])

            def emit_q(d, sc):
                emit_k(d, sc, w_sb=wq_sb, dst=qt)

            def emit_v(t):
                acc = pmisc.tile([P, DG], F32, tag="m", name="accv")
                for e in range(ET):
                    nc.tensor.matmul(
                        acc[:],
                        xs[e][t // 4][:, P * (t % 4) : P * (t % 4 + 1)],
                        wv_sb[:, e, :],
                        start=(e == 0),
                        stop=(e == ET - 1),
                    )
                nc.vector.tensor_copy(
                    v_sb[t][:, :, 0:D], acc[:].rearrange("p (h d) -> p h d", d=D)
                )

            # ---- normalization (deferred fills; dn_loc complete by then) ----
            def op_dinv(dinv_b, dn_loc):
                dn_f = dinvp.tile([HL, SC], F32, tag="dnf", name="dn_f")
                dn_i = dinvp.tile([HL, SC], F32, tag="dni", name="dn_i")
                nc.vector.tensor_copy(dn_f[:], dn_loc[:])
                nc.vector.reciprocal_approx_fast(dn_i[:], dn_f[:])
                nc.vector.tensor_copy(dinv_b[:], dn_i[:])

            def op_norm(d, at, dinv_b):
                bc = pmisc.tile([P, SC], F32, tag="m", name="bc")
                nc.tensor.matmul(
                    bc[:], sel_sb[:, P * d : P * (d + 1)], dinv_b[:],
                    start=True, stop=True,
                )
                bc_sb = workp.tile([P, SC], BF16, tag="bc")
                nc.vector.tensor_copy(bc_sb[:], bc[:])
                nc.vector.tensor_mul(at[d][:], at[d][:], bc_sb[:])

            # cc_in row order interleaves the two ranks' e-quarters so each
            # half [0:512] / [512:1024] ReduceScatters to the right slices:
            # rows = [e 0:256 | e 512:768 | e 256:512 | e 768:1024]
            CPERM = {0: 0, 1: 1, 2: 4, 3: 5, 4: 2, 5: 3, 6: 6, 7: 7}

            def op_proj(sc, et, at):
                acc = pmisc.tile([P, SC], F32, tag="m", name="acco")
                for d in range(ND):
                    nc.tensor.matmul(
                        acc[:],
                        wo_sb[:, d, P * et : P * (et + 1)],
                        at[d][:],
                        start=(d == 0),
                        stop=(d == ND - 1),
                    )
                stage = outp.tile([P, SC], BF16, tag="ot")
                nc.vector.tensor_scalar_add(stage[:], acc[:], bo_sb[:, et : et + 1])
                r = P * CPERM[et]
                nc.sync.dma_start(cc_in[sc, r : r + P, :], stage[:])

            def op_rs(sc, h):
                nc.gpsimd.collective_compute(
                    "ReduceScatter",
                    mybir.AluOpType.add,
                    replica_groups=[[0, 1], [2, 3], [4, 5], [6, 7]],
                    ins=[cc_in[sc, (E // 2) * h : (E // 2) * (h + 1), :].opt()],
                    outs=[cc_rs[sc, h].opt()],
                )
                nc.sync.dma_start(
                    outT[sc, (EH // 2) * h : (EH // 2) * (h + 1), :], cc_rs[sc, h]
                )

            def out_projection_fills(sc, at, dn_loc):
                dinv_b = dinvp.tile([HL, SC], BF16, tag="dnb", name="dinv_b")
                fills = [lambda: op_dinv(dinv_b, dn_loc)]
                for d in range(ND):
                    fills.append(lambda d=d: op_norm(d, at, dinv_b))
                for h in range(2):
                    for et in ([0, 1, 4, 5] if h == 0 else [2, 3, 6, 7]):
                        fills.append(lambda et=et: op_proj(sc, et, at))
                    fills.append(lambda h=h: op_rs(sc, h))
                return fills

            # ---- attention ----
            def attention_chunk(sc, fills):
                nt = 4 * (sc + 1)
                n_iter = ND * nt
                it = 0
                popped = 0
                at = []  # per-d AV tiles [128, 512] (bf16), normalized by fills
                dn_loc = dnp.tile([HL, SC], BF16, tag="dnl", name="dn_loc")
                for d in range(ND):
                    att = [
                        patt.tile([D + 1, SC], F32, tag="att", name=f"att{h}")
                        for h in range(2)
                    ]

                    def emit_av(j, o, ex, d=d, att=att, nt=nt):
                        for half in range(2):
                            nc.tensor.matmul(
                                att[half][:, o:SC],
                                v_sb[j][:, 2 * d + half, :],
                                ex[:, SC * half + o : SC * (half + 1)],
                                start=(j == 0),
                                stop=(j == nt - 1),
                            )

                    prev = None
                    for j in range(nt):
                        m = j - 4 * sc
                        o = P * m if m > 0 else 0
                        sco = psc.tile([P, 2 * SC], F32, tag="sc", name="sco")
                        for half in range(2):
                            r = D * half
                            nc.tensor.matmul(
                                sco[:, SC * half + o : SC * (half + 1)],
                                kt[d][j // 4][r : r + D, P * (j % 4) : P * (j % 4 + 1)],
                                qt[d][sc][r : r + D, o:SC],
                                start=True,
                                stop=True,
                                tile_position=(r, 0),
                            )
                        ex = expp.tile([P, 2 * SC], BF16, tag="ex", name="ex")
                        if o > 0:
                            for half in range(2):
                                nc.scalar.activation(
                                    ex[:, SC * half + o : SC * (half + 1)],
                                    sco[:, SC * half + o : SC * (half + 1)],
                                    mybir.ActivationFunctionType.Exp,
                                    scale=SCALE,
                                )
                        else:
                            nc.scalar.activation(
                                ex[:],
                                sco[:],
                                mybir.ActivationFunctionType.Exp,
                                scale=SCALE,
                            )
                        if m >= 0:  # diagonal tile: mask the wedge
                            for half in range(2):
                                c0 = SC * half + P * m
                                nc.vector.tensor_mul(
                                    ex[:, c0 : c0 + P], ex[:, c0 : c0 + P], mask_sb[:]
                                )
                        if prev is not None:
                            emit_av(*prev)
                        prev = (j, o, ex)
                        it += 1
                        want = (it * len(fills)) // n_iter
                        while popped < want:
                            fills[popped]()
                            popped += 1
                    emit_av(*prev)
                    # stage raw AV + denominators, release att psum
                    attn_t = attnp.tile([P, SC], BF16, tag="at", name="attn_t")
                    for half in range(2):
                        nc.vector.tensor_copy(
                            attn_t[D * half : D * (half + 1), :], att[half][0:D, :]
                        )
                        dnrow = dnp.tile([1, SC], BF16, tag="dn", name="dnrow")
                        nc.vector.tensor_copy(dnrow[:], att[half][D : D + 1, :])
                        nc.sync.dma_start(
                            dn_loc[2 * d + half : 2 * d + half + 1, :], dnrow[:]
                        )
                    at.append(attn_t)
                while popped < len(fills):
                    fills[popped]()
                    popped += 1
                return at, dn_loc

            # ---- schedule ----
            for d in range(ND):
                emit_k(d, 0)
            for d in range(ND):
                emit_q(d, 0)
            for t in range(4):
                emit_v(t)

            prev_chunk = None
            for sc in range(NSC):
                fills = []
                if sc + 1 < NSC:
                    for d in range(ND):
                        fills.append(lambda d=d, kc=sc + 1: emit_k(d, kc))
                    for t in range(4 * (sc + 1), 4 * (sc + 2)):
                        fills.append(lambda t=t: emit_v(t))
                    for d in range(ND):
                        fills.append(lambda d=d, qc=sc + 1: emit_q(d, qc))
                if prev_chunk is not None:
                    fills.extend(out_projection_fills(sc - 1, *prev_chunk))
                prev_chunk = attention_chunk(sc, fills)
            for f in out_projection_fills(NSC - 1, *prev_chunk):
                f()

    nc.compile()
    return nc


def _get_runner():
    """Build (once) and return a callable in_maps -> list of out_maps."""
    if "runner" in _CACHE:
        return _CACHE["runner"]

    nc = _build_nc()
    _CACHE["nc"] = nc

    import jax
    from jax.sharding import Mesh, PartitionSpec
    from jax.experimental.shard_map import shard_map
    from concourse import bass2jax
    from concourse.bass2jax import _bass_exec_p, partition_id_tensor

    bass2jax.install_neuronx_cc_hook()

    in_names, out_names, out_avals, zero_shapes = [], [], [], []
    partition_name = nc.partition_id_tensor.name if nc.partition_id_tensor else None
    for alloc in nc.m.functions[0].allocations:
        if not isinstance(alloc, mybir.MemoryLocationSet):
            continue
        name = alloc.memorylocations[0].name
        if alloc.kind == "ExternalInput":
            if name != partition_name:
                in_names.append(name)
        elif alloc.kind == "ExternalOutput":
            out_names.append(name)
            shape = tuple(alloc.tensor_shape)
            dtype = mybir.dt.np(alloc.dtype)
            out_avals.append(jax.core.ShapedArray(shape, dtype))
            zero_shapes.append((shape, dtype))
    n_params = len(in_names)
    all_in_names = list(in_names) + list(out_names)
    if partition_name is not None:
        all_in_names.append(partition_name)

    def _body(*args):
        operands = list(args)
        if partition_name is not None:
            operands.append(partition_id_tensor())
        outs = _bass_exec_p.bind(
            *operands,
            out_avals=tuple(out_avals),
            in_names=tuple(all_in_names),
            out_names=tuple(out_names),
            lowering_input_output_aliases=(),
            sim_require_finite=True,
            sim_require_nnan=True,
            nc=nc,
        )
        return tuple(outs)

    devices = jax.devices()[:NCORES]
    mesh = Mesh(np.asarray(devices), ("core",))
    n_outs = len(out_names)
    sharded = jax.jit(
        shard_map(
            _body,
            mesh=mesh,
            in_specs=(PartitionSpec("core"),) * (n_params + n_outs),
            out_specs=(PartitionSpec("core"),) * n_outs,
            check_rep=False,
        ),
        donate_argnums=tuple(range(n_params, n_params + n_outs)),
        keep_unused=True,
    )

    def runner(in_maps):
        per_core = [[np.asarray(m[name]) for name in in_names] for m in in_maps]
        concat_in = [
            np.concatenate([per_core[c][i] for c in range(NCORES)], axis=0)
            for i in range(n_params)
        ]
        concat_zeros = [
            np.zeros((NCORES * s[0], *s[1:]), d) for (s, d) in zero_shapes
        ]
        out_arrs = sharded(*concat_in, *concat_zeros)
        return [
            {
                name: np.asarray(out_arrs[i]).reshape(NCORES, *out_avals[i].shape)[c]
                for i, name in enumerate(out_names)
            }
            for c in range(NCORES)
        ]

    _CACHE["runner"] = runner
    _CACHE["sharded"] = sharded
    _CACHE["mesh"] = mesh
    _CACHE["meta"] = (in_names, out_names, zero_shapes)
    return runner


def timing_setup(in_maps):
    """Device-resident timing: returns (make_zeros, call).

    `call(make_zeros())` runs one on-device execution with inputs already
    resident (zeros are donated output buffers, created outside the timer).
    """
    _get_runner()
    import jax
    from jax.sharding import NamedSharding, PartitionSpec

    in_names, out_names, zero_shapes = _CACHE["meta"]
    sharding = NamedSharding(_CACHE["mesh"], PartitionSpec("core"))
    per_core = [[np.asarray(m[name]) for name in in_names] for m in in_maps]
    dev_in = [
        jax.device_put(
            np.concatenate([per_core[c][i] for c in range(NCORES)], axis=0), sharding
        )
        for i in range(len(in_names))
    ]
    jax.block_until_ready(dev_in)

    def make_zeros():
        zs = [
            jax.device_put(np.zeros((NCORES * s[0], *s[1:]), d), sharding)
            for (s, d) in zero_shapes
        ]
        jax.block_until_ready(zs)
        return zs

    def call(zs):
        out = _CACHE["sharded"](*dev_in, *zs)
        jax.block_until_ready(out)
        return out

    return make_zeros, call


def make_in_maps(x, Wq, Wk, Wv, Wo, bo):
    """Host-side sharding: slice/transpose/cast full inputs into per-core maps."""
    x = np.asarray(x, dtype=np.float32)
    Wq = np.asarray(Wq, dtype=np.float32)
    Wk = np.asarray(Wk, dtype=np.float32)
    Wv = np.asarray(Wv, dtype=np.float32)
    Wo = np.asarray(Wo, dtype=np.float32)
    bo = np.asarray(bo, dtype=np.float32)
    bf = ml_dtypes.bfloat16

    mask = np.triu(np.ones((P, P), dtype=bf))  # keep t <= s
    sel = np.zeros((HL, ND * P), dtype=bf)
    for d in range(ND):
        sel[2 * d, P * d : P * d + D] = 1
        sel[2 * d + 1, P * d + D : P * (d + 1)] = 1
    bo2 = np.ascontiguousarray((0.5 * bo).reshape(ET, P).T)  # [P, ET]
    WoT = np.ascontiguousarray(Wo.T)  # [dg_full, e]
    in_maps = []
    for c in range(NCORES):
        b, g = c // 2, c % 2
        xT = np.ascontiguousarray(x[b].T).astype(bf)  # [E, S]
        wq = np.ascontiguousarray(
            Wq[HL * g : HL * (g + 1)].transpose(1, 0, 2).reshape(E, DG)
        ).astype(bf)
        wk = np.ascontiguousarray(
            Wk[HL * g : HL * (g + 1)].transpose(1, 0, 2).reshape(E, DG)
        ).astype(bf)
        wv = np.ascontiguousarray(
            Wv[HL * g : HL * (g + 1)].transpose(1, 0, 2).reshape(E, DG)
        ).astype(bf)
        wo2 = np.ascontiguousarray(WoT[DG * g : DG * (g + 1), :]).astype(bf)
        in_maps.append(
            {
                "xT": xT,
                "wq": wq,
                "wk": wk,
                "wv": wv,
                "wo2": wo2,
                "bo2": bo2,
                "mask": mask,
                "sel8": sel,
            }
        )
    return in_maps


def assemble_output(results):
    """Gather per-core outT [EH, S] slices into the full [B, S, E] output."""
    out = np.empty((B, S, E), dtype=np.float32)
    for c in range(NCORES):
        b, g = c // 2, c % 2
        o = results[c]["outT"]  # [NSC, EH, SC]
        out[b, :, EH * g : EH * (g + 1)] = (
            o.transpose(0, 2, 1).reshape(S, EH).astype(np.float32)
        )
    return out


def kernel(x, Wq, Wk, Wv, Wo, bo):
    runner = _get_runner()
    in_maps = make_in_maps(x, Wq, Wk, Wv, Wo, bo)
    results = runner(in_maps)
    return assemble_output(results)


# revision 38
# speedup vs baseline: 1.1264x; 1.0052x over previous
"""Causal multi-head attention on 8 Trainium2 NeuronCores.

Problem: x[B=4,S=2048,E=1024], Wq/Wk/Wv[H=16,E,D=64], Wo[E,E], bo[E].
  out = softmax_causal(q k^T / sqrt(D)) v, heads concat, @ Wo.T + bo

Sharding (tensor parallel over heads, data parallel over batch):
  core c -> (batch b = c//2, head-group g = c%2 of 8 heads).
  Each core: QKV projections + attention for its 8 heads of its batch,
  normalization, and a FULL-width output-projection partial from its own
  heads (Wo split on its input axis); a pairwise ReduceScatter(add) over
  (2b, 2b+1) sums the partials and leaves each core its 512 output
  columns, written straight into the kernel output.  No peer data is ever
  needed by compute -> no collective on the critical path except the
  final add.

Kernel internals (per core), v3:
  - All SBUF data bf16 (psum f32).  Activations transposed: xT[E,S],
    QT/KT[dg,S], scoresT[t,q]; softmax denominator from a ones-column
    appended to V; probabilities feed the AV matmul moving operand.
  - Scores for a head PAIR (row-tiled 64-contraction matmuls at PE row
    groups 0/64) share one [128, 1024] psum tile; ONE exp activation
    covers both heads.
  - Causality at 128 granularity: suffix-restricted scores/exp/AV on
    diagonal tiles + a [128,128] upper-tri wedge multiply.
  - 1/denom via reciprocal_approx_fast on a per-chunk [8,512] stack
    (gathered by tiny SBUF->SBUF DMAs); broadcast across each head's 64
    rows by a small selector matmul; bf16 in-place normalize.
  - Phase-1 (QKV) and out-projection work interleaved into the attention
    emission so TensorE always has dense work while ScalarE exp runs
    (HAM stays at 2.4 GHz).  x is loaded in [128,512] column slices so
    the prologue is never DMA-starved.
"""

import os
import sys

for _p in ("/opt/trn_rl_repo", "/root/.axon_site/_ro/trn_rl_repo"):
    if os.path.isdir(_p) and _p not in sys.path:
        sys.path.append(_p)

import numpy as np
import ml_dtypes

import concourse.bass as bass
import concourse.mybir as mybir
import concourse.tile as tile
from concourse import bacc

B, S, E, H, D = 4, 2048, 1024, 16, 64
NCORES = 8
G = 2  # head groups
HL = H // G  # heads per core = 8
DG = HL * D  # local head dim = 512
EH = E // G  # final output columns per core = 512
P = 128
SC = 512  # sequence chunk
NSC = S // SC  # 4
NT = S // P  # 16 key tiles
ET = E // P  # 8 embedding tiles
ND = DG // P  # 4 head-pairs per core
SCALE = 1.0 / np.sqrt(D)

F32 = mybir.dt.float32
BF16 = mybir.dt.bfloat16

_CACHE = {}


def _build_nc():
    nc = bacc.Bacc("TRN2", target_bir_lowering=False, debug=False, num_devices=NCORES)

    xT = nc.dram_tensor("xT", [E, S], BF16, kind="ExternalInput")
    wq = nc.dram_tensor("wq", [E, DG], BF16, kind="ExternalInput")
    wk = nc.dram_tensor("wk", [E, DG], BF16, kind="ExternalInput")
    wv = nc.dram_tensor("wv", [E, DG], BF16, kind="ExternalInput")
    wo2 = nc.dram_tensor("wo2", [DG, E], BF16, kind="ExternalInput")
    bo2 = nc.dram_tensor("bo2", [P, ET], F32, kind="ExternalInput")
    mask = nc.dram_tensor("mask", [P, P], BF16, kind="ExternalInput")
    sel8 = nc.dram_tensor("sel8", [HL, ND * P], BF16, kind="ExternalInput")
    outT = nc.dram_tensor("outT", [NSC, EH, SC], BF16, kind="ExternalOutput")

    with tile.TileContext(nc) as tc:
        with (
            tc.tile_pool(name="persist", bufs=1) as persist,
            tc.tile_pool(name="expp", bufs=4) as expp,
            tc.tile_pool(name="attnp", bufs=16) as attnp,
            tc.tile_pool(name="dnp", bufs=5) as dnp,
            tc.tile_pool(name="dinvp", bufs=3) as dinvp,
            tc.tile_pool(name="workp", bufs=4) as workp,
            tc.tile_pool(name="outp", bufs=3) as outp,
            tc.tile_pool(name="psc", bufs=2, space="PSUM") as psc,
            tc.tile_pool(name="patt", bufs=2, space="PSUM") as patt,
            tc.tile_pool(name="pmisc", bufs=2, space="PSUM") as pmisc,
            tc.tile_pool(name="dram", bufs=1, space="DRAM") as dram,
        ):
            # ---- persistent tiles ----
            xs = [
                [persist.tile([P, SC], BF16, name=f"x{e}_{c}") for c in range(NSC)]
                for e in range(ET)
            ]
            wq_sb = persist.tile([P, ET, DG], BF16, name="wq")
            wk_sb = persist.tile([P, ET, DG], BF16, name="wk")
            wv_sb = persist.tile([P, ET, DG], BF16, name="wv")
            wo_sb = persist.tile([P, ND, E], BF16, name="wo")
            bo_sb = persist.tile([P, ET], F32, name="bo")
            mask_sb = persist.tile([P, P], BF16, name="mask")
            sel_sb = persist.tile([HL, ND * P], BF16, name="sel")
            kt = [
                [persist.tile([P, SC], BF16, name=f"kt{d}_{kc}") for kc in range(NSC)]
                for d in range(ND)
            ]
            qt = [
                [persist.tile([P, SC], BF16, name=f"qt{d}_{sc}") for sc in range(NSC)]
                for d in range(ND)
            ]
            v_sb = [persist.tile([P, HL, D + 1], BF16, name=f"v{t}") for t in range(NT)]

            cc_in = dram.tile([NSC, E, SC], BF16)
            cc_rs = dram.tile([NSC, 2, EH // 2, SC], BF16)

            # ---- input DMAs: per-e weight blocks interleaved with x slices
            # so K(0)'s e-th matmul can start as soon as its operands land
            for e in range(ET):
                nc.sync.dma_start(
                    wk_sb[:, e, :], wk[P * e : P * (e + 1), :]
                )
                nc.sync.dma_start(xs[e][0][:], xT[P * e : P * (e + 1), 0:SC])
            for e in range(ET):
                nc.sync.dma_start(wq_sb[:, e, :], wq[P * e : P * (e + 1), :])
            for e in range(ET):
                nc.sync.dma_start(wv_sb[:, e, :], wv[P * e : P * (e + 1), :])
            for c in range(1, NSC):
                for e in range(ET):
                    nc.sync.dma_start(
                        xs[e][c][:], xT[P * e : P * (e + 1), SC * c : SC * (c + 1)]
                    )
            nc.sync.dma_start(wo_sb[:], wo2.rearrange("(ko p) m -> p ko m", p=P))
            nc.sync.dma_start(bo_sb[:], bo2[:])
            nc.sync.dma_start(mask_sb[:], mask[:])
            nc.sync.dma_start(sel_sb[:], sel8[:])
            for t in range(NT):
                nc.vector.memset(v_sb[t][:, :, D], 1.0)

            # ---- phase-1 emitters ----
            def emit_k(d, kc, w_sb=wk_sb, dst=kt):
                acc = pmisc.tile([P, SC], F32, tag="m", name="acc")
                for e in range(ET):
                    nc.tensor.matmul(
                        acc[:],
                        w_sb[:, e, P * d : P * (d + 1)],
                        xs[e][kc][:],
                        start=(e == 0),
                        stop=(e == ET - 1),
                    )
                nc.vector.tensor_copy(dst[d][kc][:], acc[:])

            def emit_q(d, sc):
                emit_k(d, sc, w_sb=wq_sb, dst=qt)

            def emit_v(t):
                acc = pmisc.tile([P, DG], F32, tag="m", name="accv")
                for e in range(ET):
                    nc.tensor.matmul(
                        acc[:],
                        xs[e][t // 4][:, P * (t % 4) : P * (t % 4 + 1)],
                        wv_sb[:, e, :],
                        start=(e == 0),
                        stop=(e == ET - 1),
                    )
                nc.vector.tensor_copy(
                    v_sb[t][:, :, 0:D], acc[:].rearrange("p (h d) -> p h d", d=D)
                )

            # ---- normalization (deferred fills; dn_loc complete by then) ----
            def op_dinv(dinv_b, dn_loc, sce=False):
                # sce: route casts via ScalarE (idle in the epilogue)
                cp = nc.scalar.copy if sce else nc.vector.tensor_copy
                dn_f = dinvp.tile([HL, SC], F32, tag="dnf", name="dn_f")
                dn_i = dinvp.tile([HL, SC], F32, tag="dni", name="dn_i")
                cp(dn_f[:], dn_loc[:])
                nc.vector.reciprocal_approx_fast(dn_i[:], dn_f[:])
                cp(dinv_b[:], dn_i[:])

            def op_norm(d, at, dinv_b, sce=False):
                bc = pmisc.tile([P, SC], F32, tag="m", name="bc")
                nc.tensor.matmul(
                    bc[:], sel_sb[:, P * d : P * (d + 1)], dinv_b[:],
                    start=True, stop=True,
                )
                bc_sb = workp.tile([P, SC], BF16, tag="bc")
                (nc.scalar.copy if sce else nc.vector.tensor_copy)(bc_sb[:], bc[:])
                nc.vector.tensor_mul(at[d][:], at[d][:], bc_sb[:])

            # cc_in row order interleaves the two ranks' e-quarters so each
            # half [0:512] / [512:1024] ReduceScatters to the right slices:
            # rows = [e 0:256 | e 512:768 | e 256:512 | e 768:1024]
            CPERM = {0: 0, 1: 1, 2: 4, 3: 5, 4: 2, 5: 3, 6: 6, 7: 7}

            def op_proj(sc, et, at):
                acc = pmisc.tile([P, SC], F32, tag="m", name="acco")
                for d in range(ND):
                    nc.tensor.matmul(
                        acc[:],
                        wo_sb[:, d, P * et : P * (et + 1)],
                        at[d][:],
                        start=(d == 0),
                        stop=(d == ND - 1),
                    )
                stage = outp.tile([P, SC], BF16, tag="ot")
                nc.vector.tensor_scalar_add(stage[:], acc[:], bo_sb[:, et : et + 1])
                r = P * CPERM[et]
                nc.sync.dma_start(cc_in[sc, r : r + P, :], stage[:])

            def op_rs(sc, h):
                nc.gpsimd.collective_compute(
                    "ReduceScatter",
                    mybir.AluOpType.add,
                    replica_groups=[[0, 1], [2, 3], [4, 5], [6, 7]],
                    ins=[cc_in[sc, (E // 2) * h : (E // 2) * (h + 1), :].opt()],
                    outs=[cc_rs[sc, h].opt()],
                )
                nc.sync.dma_start(
                    outT[sc, (EH // 2) * h : (EH // 2) * (h + 1), :], cc_rs[sc, h]
                )

            def out_projection_fills(sc, at, dn_loc, sce=False):
                dinv_b = dinvp.tile([HL, SC], BF16, tag="dnb", name="dinv_b")
                fills = [lambda: op_dinv(dinv_b, dn_loc, sce)]
                for d in range(ND):
                    fills.append(lambda d=d: op_norm(d, at, dinv_b, sce))
                for h in range(2):
                    for et in ([0, 1, 4, 5] if h == 0 else [2, 3, 6, 7]):
                        fills.append(lambda et=et: op_proj(sc, et, at))
                    fills.append(lambda h=h: op_rs(sc, h))
                return fills

            # ---- attention ----
            def attention_chunk(sc, fills):
                nt = 4 * (sc + 1)
                n_iter = ND * nt
                it = 0
                popped = 0
                at = []  # per-d AV tiles [128, 512] (bf16), normalized by fills
                dn_loc = dnp.tile([HL, SC], BF16, tag="dnl", name="dn_loc")
                for d in range(ND):
                    att = [
                        patt.tile([D + 1, SC], F32, tag="att", name=f"att{h}")
                        for h in range(2)
                    ]

                    def emit_av(j, o, ex, d=d, att=att, nt=nt):
                        for half in range(2):
                            nc.tensor.matmul(
                                att[half][:, o:SC],
                                v_sb[j][:, 2 * d + half, :],
                                ex[:, SC * half + o : SC * (half + 1)],
                                start=(j == 0),
                                stop=(j == nt - 1),
                            )

                    prev = None
                    for j in range(nt):
                        m = j - 4 * sc
                        o = P * m if m > 0 else 0
                        sco = psc.tile([P, 2 * SC], F32, tag="sc", name="sco")
                        for half in range(2):
                            r = D * half
                            nc.tensor.matmul(
                                sco[:, SC * half + o : SC * (half + 1)],
                                kt[d][j // 4][r : r + D, P * (j % 4) : P * (j % 4 + 1)],
                                qt[d][sc][r : r + D, o:SC],
                                start=True,
                                stop=True,
                                tile_position=(r, 0),
                            )
                        ex = expp.tile([P, 2 * SC], BF16, tag="ex", name="ex")
                        if o > 0:
                            for half in range(2):
                                nc.scalar.activation(
                                    ex[:, SC * half + o : SC * (half + 1)],
                                    sco[:, SC * half + o : SC * (half + 1)],
                                    mybir.ActivationFunctionType.Exp,
                                    scale=SCALE,
                                )
                        else:
                            nc.scalar.activation(
                                ex[:],
                                sco[:],
                                mybir.ActivationFunctionType.Exp,
                                scale=SCALE,
                            )
                        if m >= 0:  # diagonal tile: mask the wedge
                            for half in range(2):
                                c0 = SC * half + P * m
                                nc.vector.tensor_mul(
                                    ex[:, c0 : c0 + P], ex[:, c0 : c0 + P], mask_sb[:]
                                )
                        if prev is not None:
                            emit_av(*prev)
                        prev = (j, o, ex)
                        it += 1
                        want = (it * len(fills)) // n_iter
                        while popped < want:
                            fills[popped]()
                            popped += 1
                    emit_av(*prev)
                    # stage raw AV + denominators, release att psum
                    attn_t = attnp.tile([P, SC], BF16, tag="at", name="attn_t")
                    for half in range(2):
                        nc.vector.tensor_copy(
                            attn_t[D * half : D * (half + 1), :], att[half][0:D, :]
                        )
                        dnrow = dnp.tile([1, SC], BF16, tag="dn", name="dnrow")
                        nc.vector.tensor_copy(dnrow[:], att[half][D : D + 1, :])
                        nc.sync.dma_start(
                            dn_loc[2 * d + half : 2 * d + half + 1, :], dnrow[:]
                        )
                    at.append(attn_t)
                while popped < len(fills):
                    fills[popped]()
                    popped += 1
                return at, dn_loc

            # ---- schedule ----
            for d in range(ND):
                emit_k(d, 0)
            for d in range(ND):
                emit_q(d, 0)
            for t in range(4):
                emit_v(t)

            chunks = {}
            for sc in range(NSC):
                fills = []
                if sc + 1 < NSC:
                    # Q first: chunk sc+1 needs it immediately; K/V tiles of
                    # key-chunk sc+1 aren't touched until late in sc+1
                    for d in range(ND):
                        fills.append(lambda d=d, qc=sc + 1: emit_q(d, qc))
                    for d in range(ND):
                        fills.append(lambda d=d, kc=sc + 1: emit_k(d, kc))
                    for t in range(4 * (sc + 1), 4 * (sc + 2)):
                        fills.append(lambda t=t: emit_v(t))
                # out-projections shifted late: the tail chunks are the
                # ACT-bound ones that need PE fill work
                if sc == 2:
                    fills.extend(out_projection_fills(0, *chunks[0]))
                if sc == 3:
                    fills.extend(out_projection_fills(1, *chunks[1]))
                    fills.extend(out_projection_fills(2, *chunks[2]))
                chunks[sc] = attention_chunk(sc, fills)
            for f in out_projection_fills(NSC - 1, *chunks[NSC - 1], sce=True):
                f()

    nc.compile()
    return nc


def _get_runner():
    """Build (once) and return a callable in_maps -> list of out_maps."""
    if "runner" in _CACHE:
        return _CACHE["runner"]

    nc = _build_nc()
    _CACHE["nc"] = nc

    import jax
    from jax.sharding import Mesh, PartitionSpec
    from jax.experimental.shard_map import shard_map
    from concourse import bass2jax
    from concourse.bass2jax import _bass_exec_p, partition_id_tensor

    bass2jax.install_neuronx_cc_hook()

    in_names, out_names, out_avals, zero_shapes = [], [], [], []
    partition_name = nc.partition_id_tensor.name if nc.partition_id_tensor else None
    for alloc in nc.m.functions[0].allocations:
        if not isinstance(alloc, mybir.MemoryLocationSet):
            continue
        name = alloc.memorylocations[0].name
        if alloc.kind == "ExternalInput":
            if name != partition_name:
                in_names.append(name)
        elif alloc.kind == "ExternalOutput":
            out_names.append(name)
            shape = tuple(alloc.tensor_shape)
            dtype = mybir.dt.np(alloc.dtype)
            out_avals.append(jax.core.ShapedArray(shape, dtype))
            zero_shapes.append((shape, dtype))
    n_params = len(in_names)
    all_in_names = list(in_names) + list(out_names)
    if partition_name is not None:
        all_in_names.append(partition_name)

    def _body(*args):
        operands = list(args)
        if partition_name is not None:
            operands.append(partition_id_tensor())
        outs = _bass_exec_p.bind(
            *operands,
            out_avals=tuple(out_avals),
            in_names=tuple(all_in_names),
            out_names=tuple(out_names),
            lowering_input_output_aliases=(),
            sim_require_finite=True,
            sim_require_nnan=True,
            nc=nc,
        )
        return tuple(outs)

    devices = jax.devices()[:NCORES]
    mesh = Mesh(np.asarray(devices), ("core",))
    n_outs = len(out_names)
    sharded = jax.jit(
        shard_map(
            _body,
            mesh=mesh,
            in_specs=(PartitionSpec("core"),) * (n_params + n_outs),
            out_specs=(PartitionSpec("core"),) * n_outs,
            check_rep=False,
        ),
        donate_argnums=tuple(range(n_params, n_params + n_outs)),
        keep_unused=True,
    )

    def runner(in_maps):
        per_core = [[np.asarray(m[name]) for name in in_names] for m in in_maps]
        concat_in = [
            np.concatenate([per_core[c][i] for c in range(NCORES)], axis=0)
            for i in range(n_params)
        ]
        concat_zeros = [
            np.zeros((NCORES * s[0], *s[1:]), d) for (s, d) in zero_shapes
        ]
        out_arrs = sharded(*concat_in, *concat_zeros)
        return [
            {
                name: np.asarray(out_arrs[i]).reshape(NCORES, *out_avals[i].shape)[c]
                for i, name in enumerate(out_names)
            }
            for c in range(NCORES)
        ]

    _CACHE["runner"] = runner
    _CACHE["sharded"] = sharded
    _CACHE["mesh"] = mesh
    _CACHE["meta"] = (in_names, out_names, zero_shapes)
    return runner


def timing_setup(in_maps):
    """Device-resident timing: returns (make_zeros, call).

    `call(make_zeros())` runs one on-device execution with inputs already
    resident (zeros are donated output buffers, created outside the timer).
    """
    _get_runner()
    import jax
    from jax.sharding import NamedSharding, PartitionSpec

    in_names, out_names, zero_shapes = _CACHE["meta"]
    sharding = NamedSharding(_CACHE["mesh"], PartitionSpec("core"))
    per_core = [[np.asarray(m[name]) for name in in_names] for m in in_maps]
    dev_in = [
        jax.device_put(
            np.concatenate([per_core[c][i] for c in range(NCORES)], axis=0), sharding
        )
        for i in range(len(in_names))
    ]
    jax.block_until_ready(dev_in)

    def make_zeros():
        zs = [
            jax.device_put(np.zeros((NCORES * s[0], *s[1:]), d), sharding)
            for (s, d) in zero_shapes
        ]
        jax.block_until_ready(zs)
        return zs

    def call(zs):
        out = _CACHE["sharded"](*dev_in, *zs)
        jax.block_until_ready(out)
        return out

    return make_zeros, call


def make_in_maps(x, Wq, Wk, Wv, Wo, bo):
    """Host-side sharding: slice/transpose/cast full inputs into per-core maps."""
    x = np.asarray(x, dtype=np.float32)
    Wq = np.asarray(Wq, dtype=np.float32)
    Wk = np.asarray(Wk, dtype=np.float32)
    Wv = np.asarray(Wv, dtype=np.float32)
    Wo = np.asarray(Wo, dtype=np.float32)
    bo = np.asarray(bo, dtype=np.float32)
    bf = ml_dtypes.bfloat16

    mask = np.triu(np.ones((P, P), dtype=bf))  # keep t <= s
    sel = np.zeros((HL, ND * P), dtype=bf)
    for d in range(ND):
        sel[2 * d, P * d : P * d + D] = 1
        sel[2 * d + 1, P * d + D : P * (d + 1)] = 1
    bo2 = np.ascontiguousarray((0.5 * bo).reshape(ET, P).T)  # [P, ET]
    WoT = np.ascontiguousarray(Wo.T)  # [dg_full, e]
    in_maps = []
    for c in range(NCORES):
        b, g = c // 2, c % 2
        xT = np.ascontiguousarray(x[b].T).astype(bf)  # [E, S]
        wq = np.ascontiguousarray(
            Wq[HL * g : HL * (g + 1)].transpose(1, 0, 2).reshape(E, DG)
        ).astype(bf)
        wk = np.ascontiguousarray(
            Wk[HL * g : HL * (g + 1)].transpose(1, 0, 2).reshape(E, DG)
        ).astype(bf)
        wv = np.ascontiguousarray(
            Wv[HL * g : HL * (g + 1)].transpose(1, 0, 2).reshape(E, DG)
        ).astype(bf)
        wo2 = np.ascontiguousarray(WoT[DG * g : DG * (g + 1), :]).astype(bf)
        in_maps.append(
            {
                "xT": xT,
                "wq": wq,
                "wk": wk,
                "wv": wv,
                "wo2": wo2,
                "bo2": bo2,
                "mask": mask,
                "sel8": sel,
            }
        )
    return in_maps


def assemble_output(results):
    """Gather per-core outT [EH, S] slices into the full [B, S, E] output."""
    out = np.empty((B, S, E), dtype=np.float32)
    for c in range(NCORES):
        b, g = c // 2, c % 2
        o = results[c]["outT"]  # [NSC, EH, SC]
        out[b, :, EH * g : EH * (g + 1)] = (
            o.transpose(0, 2, 1).reshape(S, EH).astype(np.float32)
        )
    return out


def kernel(x, Wq, Wk, Wv, Wo, bo):
    runner = _get_runner()
    in_maps = make_in_maps(x, Wq, Wk, Wv, Wo, bo)
    results = runner(in_maps)
    return assemble_output(results)


# revision 45
# speedup vs baseline: 1.1297x; 1.0029x over previous
"""Causal multi-head attention on 8 Trainium2 NeuronCores.

Problem: x[B=4,S=2048,E=1024], Wq/Wk/Wv[H=16,E,D=64], Wo[E,E], bo[E].
  out = softmax_causal(q k^T / sqrt(D)) v, heads concat, @ Wo.T + bo

Sharding (tensor parallel over heads, data parallel over batch):
  core c -> (batch b = c//2, head-group g = c%2 of 8 heads).
  Each core: QKV projections + attention for its 8 heads of its batch,
  normalization, and a FULL-width output-projection partial from its own
  heads (Wo split on its input axis); a pairwise ReduceScatter(add) over
  (2b, 2b+1) sums the partials and leaves each core its 512 output
  columns, written straight into the kernel output.  No peer data is ever
  needed by compute -> no collective on the critical path except the
  final add.

Kernel internals (per core), v3:
  - All SBUF data bf16 (psum f32).  Activations transposed: xT[E,S],
    QT/KT[dg,S], scoresT[t,q]; softmax denominator from a ones-column
    appended to V; probabilities feed the AV matmul moving operand.
  - Scores for a head PAIR (row-tiled 64-contraction matmuls at PE row
    groups 0/64) share one [128, 1024] psum tile; ONE exp activation
    covers both heads.
  - Causality at 128 granularity: suffix-restricted scores/exp/AV on
    diagonal tiles + a [128,128] upper-tri wedge multiply.
  - 1/denom via reciprocal_approx_fast on a per-chunk [8,512] stack
    (gathered by tiny SBUF->SBUF DMAs); broadcast across each head's 64
    rows by a small selector matmul; bf16 in-place normalize.
  - Phase-1 (QKV) and out-projection work interleaved into the attention
    emission so TensorE always has dense work while ScalarE exp runs
    (HAM stays at 2.4 GHz).  x is loaded in [128,512] column slices so
    the prologue is never DMA-starved.
"""

import os
import sys

for _p in ("/opt/trn_rl_repo", "/root/.axon_site/_ro/trn_rl_repo"):
    if os.path.isdir(_p) and _p not in sys.path:
        sys.path.append(_p)

import numpy as np
import ml_dtypes

import concourse.bass as bass
import concourse.mybir as mybir
import concourse.tile as tile
from concourse import bacc

B, S, E, H, D = 4, 2048, 1024, 16, 64
NCORES = 8
G = 2  # head groups
HL = H // G  # heads per core = 8
DG = HL * D  # local head dim = 512
EH = E // G  # final output columns per core = 512
P = 128
SC = 512  # sequence chunk
NSC = S // SC  # 4
NT = S // P  # 16 key tiles
ET = E // P  # 8 embedding tiles
ND = DG // P  # 4 head-pairs per core
SCALE = 1.0 / np.sqrt(D)

F32 = mybir.dt.float32
BF16 = mybir.dt.bfloat16

_CACHE = {}


def _build_nc():
    nc = bacc.Bacc("TRN2", target_bir_lowering=False, debug=False, num_devices=NCORES)

    xT = nc.dram_tensor("xT", [E, S], BF16, kind="ExternalInput")
    wq = nc.dram_tensor("wq", [E, DG], BF16, kind="ExternalInput")
    wk = nc.dram_tensor("wk", [E, DG], BF16, kind="ExternalInput")
    wv = nc.dram_tensor("wv", [E, DG], BF16, kind="ExternalInput")
    wo2 = nc.dram_tensor("wo2", [DG, E], BF16, kind="ExternalInput")
    bo2 = nc.dram_tensor("bo2", [P, ET], F32, kind="ExternalInput")
    mask = nc.dram_tensor("mask", [P, P], BF16, kind="ExternalInput")
    sel8 = nc.dram_tensor("sel8", [HL, ND * P], BF16, kind="ExternalInput")
    outT = nc.dram_tensor("outT", [NSC, EH, SC], BF16, kind="ExternalOutput")

    with tile.TileContext(nc) as tc:
        with (
            tc.tile_pool(name="persist", bufs=1) as persist,
            tc.tile_pool(name="expp", bufs=6) as expp,
            tc.tile_pool(name="attnp", bufs=16) as attnp,
            tc.tile_pool(name="dnp", bufs=5) as dnp,
            tc.tile_pool(name="dinvp", bufs=3) as dinvp,
            tc.tile_pool(name="workp", bufs=6) as workp,
            tc.tile_pool(name="outp", bufs=4) as outp,
            tc.tile_pool(name="psc", bufs=2, space="PSUM") as psc,
            tc.tile_pool(name="patt", bufs=2, space="PSUM") as patt,
            tc.tile_pool(name="pmisc", bufs=2, space="PSUM") as pmisc,
            tc.tile_pool(name="dram", bufs=1, space="DRAM") as dram,
        ):
            # ---- persistent tiles ----
            xs = [
                [persist.tile([P, SC], BF16, name=f"x{e}_{c}") for c in range(NSC)]
                for e in range(ET)
            ]
            wq_sb = persist.tile([P, ET, DG], BF16, name="wq")
            wk_sb = persist.tile([P, ET, DG], BF16, name="wk")
            wv_sb = persist.tile([P, ET, DG], BF16, name="wv")
            wo_sb = persist.tile([P, ND, E], BF16, name="wo")
            bo_sb = persist.tile([P, ET], F32, name="bo")
            mask_sb = persist.tile([P, P], BF16, name="mask")
            sel_sb = persist.tile([HL, ND * P], BF16, name="sel")
            kt = [
                [persist.tile([P, SC], BF16, name=f"kt{d}_{kc}") for kc in range(NSC)]
                for d in range(ND)
            ]
            qt = [
                [persist.tile([P, SC], BF16, name=f"qt{d}_{sc}") for sc in range(NSC)]
                for d in range(ND)
            ]
            v_sb = [persist.tile([P, HL, D + 1], BF16, name=f"v{t}") for t in range(NT)]

            cc_in = dram.tile([NSC, E, SC], BF16)
            cc_rs = dram.tile([NSC, 2, EH // 2, SC], BF16)
            # last chunk: query-halved layout so its two ReduceScatters
            # pipeline with the projection instead of fully serializing
            cc_q = dram.tile([2, E, SC // 2], BF16)
            cc_rs_q = dram.tile([2, EH, SC // 2], BF16)

            # ---- input DMAs: per-e weight blocks interleaved with x slices
            # so K(0)'s e-th matmul can start as soon as its operands land
            for e in range(ET):
                nc.sync.dma_start(
                    wk_sb[:, e, :], wk[P * e : P * (e + 1), :]
                )
                nc.sync.dma_start(xs[e][0][:], xT[P * e : P * (e + 1), 0:SC])
            for e in range(ET):
                nc.sync.dma_start(wq_sb[:, e, :], wq[P * e : P * (e + 1), :])
            for e in range(ET):
                nc.sync.dma_start(wv_sb[:, e, :], wv[P * e : P * (e + 1), :])
            for c in range(1, NSC):
                for e in range(ET):
                    nc.sync.dma_start(
                        xs[e][c][:], xT[P * e : P * (e + 1), SC * c : SC * (c + 1)]
                    )
            nc.sync.dma_start(wo_sb[:], wo2.rearrange("(ko p) m -> p ko m", p=P))
            nc.sync.dma_start(bo_sb[:], bo2[:])
            nc.sync.dma_start(mask_sb[:], mask[:])
            nc.sync.dma_start(sel_sb[:], sel8[:])
            for t in range(NT):
                nc.vector.memset(v_sb[t][:, :, D], 1.0)

            # ---- phase-1 emitters ----
            def emit_k(d, kc, w_sb=wk_sb, dst=kt):
                acc = pmisc.tile([P, SC], F32, tag="m", name="acc")
                for e in range(ET):
                    nc.tensor.matmul(
                        acc[:],
                        w_sb[:, e, P * d : P * (d + 1)],
                        xs[e][kc][:],
                        start=(e == 0),
                        stop=(e == ET - 1),
                    )
                nc.vector.tensor_copy(dst[d][kc][:], acc[:])

            def emit_q(d, sc):
                emit_k(d, sc, w_sb=wq_sb, dst=qt)

            def emit_v(t):
                acc = pmisc.tile([P, DG], F32, tag="m", name="accv")
                for e in range(ET):
                    nc.tensor.matmul(
                        acc[:],
                        xs[e][t // 4][:, P * (t % 4) : P * (t % 4 + 1)],
                        wv_sb[:, e, :],
                        start=(e == 0),
                        stop=(e == ET - 1),
                    )
                nc.vector.tensor_copy(
                    v_sb[t][:, :, 0:D], acc[:].rearrange("p (h d) -> p h d", d=D)
                )

            # ---- normalization (deferred fills; dn_loc complete by then) ----
            def op_dinv(dinv_b, dn_loc, sce=False):
                # sce: route casts via ScalarE (idle in the epilogue)
                cp = nc.scalar.copy if sce else nc.vector.tensor_copy
                dn_f = dinvp.tile([HL, SC], F32, tag="dnf", name="dn_f")
                dn_i = dinvp.tile([HL, SC], F32, tag="dni", name="dn_i")
                cp(dn_f[:], dn_loc[:])
                nc.vector.reciprocal_approx_fast(dn_i[:], dn_f[:])
                cp(dinv_b[:], dn_i[:])

            def op_norm(d, at, dinv_b, sce=False):
                bc = pmisc.tile([P, SC], F32, tag="m", name="bc")
                nc.tensor.matmul(
                    bc[:], sel_sb[:, P * d : P * (d + 1)], dinv_b[:],
                    start=True, stop=True,
                )
                bc_sb = workp.tile([P, SC], BF16, tag="bc")
                (nc.scalar.copy if sce else nc.vector.tensor_copy)(bc_sb[:], bc[:])
                nc.vector.tensor_mul(at[d][:], at[d][:], bc_sb[:])

            # cc_in row order interleaves the two ranks' e-quarters so each
            # half [0:512] / [512:1024] ReduceScatters to the right slices:
            # rows = [e 0:256 | e 512:768 | e 256:512 | e 768:1024]
            CPERM = {0: 0, 1: 1, 2: 4, 3: 5, 4: 2, 5: 3, 6: 6, 7: 7}

            def op_proj(sc, et, at):
                acc = pmisc.tile([P, SC], F32, tag="m", name="acco")
                for d in range(ND):
                    nc.tensor.matmul(
                        acc[:],
                        wo_sb[:, d, P * et : P * (et + 1)],
                        at[d][:],
                        start=(d == 0),
                        stop=(d == ND - 1),
                    )
                stage = outp.tile([P, SC], BF16, tag="ot")
                nc.vector.tensor_scalar_add(stage[:], acc[:], bo_sb[:, et : et + 1])
                r = P * CPERM[et]
                nc.sync.dma_start(cc_in[sc, r : r + P, :], stage[:])

            def op_rs(sc, h):
                nc.gpsimd.collective_compute(
                    "ReduceScatter",
                    mybir.AluOpType.add,
                    replica_groups=[[0, 1], [2, 3], [4, 5], [6, 7]],
                    ins=[cc_in[sc, (E // 2) * h : (E // 2) * (h + 1), :].opt()],
                    outs=[cc_rs[sc, h].opt()],
                )
                nc.sync.dma_start(
                    outT[sc, (EH // 2) * h : (EH // 2) * (h + 1), :], cc_rs[sc, h]
                )

            def op_proj_q(et, qh, at):
                q0 = (SC // 2) * qh
                acc = pmisc.tile([P, SC // 2], F32, tag="m", name="accq")
                for d in range(ND):
                    nc.tensor.matmul(
                        acc[:],
                        wo_sb[:, d, P * et : P * (et + 1)],
                        at[d][:, q0 : q0 + SC // 2],
                        start=(d == 0),
                        stop=(d == ND - 1),
                    )
                stage = outp.tile([P, SC // 2], BF16, tag="otq")
                nc.vector.tensor_scalar_add(stage[:], acc[:], bo_sb[:, et : et + 1])
                # single 1024-row ReduceScatter: rank r takes rows [512r:512r+512],
                # so et blocks stay in identity order (no CPERM here)
                nc.sync.dma_start(cc_q[qh, P * et : P * (et + 1), :], stage[:])

            def op_rs_q(sc, qh):
                nc.gpsimd.collective_compute(
                    "ReduceScatter",
                    mybir.AluOpType.add,
                    replica_groups=[[0, 1], [2, 3], [4, 5], [6, 7]],
                    ins=[cc_q[qh].opt()],
                    outs=[cc_rs_q[qh].opt()],
                )
                q0 = (SC // 2) * qh
                nc.sync.dma_start(outT[sc, :, q0 : q0 + SC // 2], cc_rs_q[qh])

            def out_projection_fills(sc, at, dn_loc, sce=False, qsplit=False):
                dinv_b = dinvp.tile([HL, SC], BF16, tag="dnb", name="dinv_b")
                fills = [lambda: op_dinv(dinv_b, dn_loc, sce)]
                for d in range(ND):
                    fills.append(lambda d=d: op_norm(d, at, dinv_b, sce))
                if qsplit:
                    for qh in range(2):
                        for et in range(ET):
                            fills.append(lambda et=et, qh=qh: op_proj_q(et, qh, at))
                        fills.append(lambda qh=qh: op_rs_q(sc, qh))
                else:
                    for h in range(2):
                        for et in ([0, 1, 4, 5] if h == 0 else [2, 3, 6, 7]):
                            fills.append(lambda et=et: op_proj(sc, et, at))
                        fills.append(lambda h=h: op_rs(sc, h))
                return fills

            # ---- attention ----
            def attention_chunk(sc, fills):
                nt = 4 * (sc + 1)
                n_iter = ND * nt
                it = 0
                popped = 0
                at = []  # per-d AV tiles [128, 512] (bf16), normalized by fills
                dn_loc = dnp.tile([HL, SC], BF16, tag="dnl", name="dn_loc")
                for d in range(ND):
                    att = [
                        patt.tile([D + 1, SC], F32, tag="att", name=f"att{h}")
                        for h in range(2)
                    ]

                    def emit_av(j, o, ex, d=d, att=att, nt=nt):
                        for half in range(2):
                            nc.tensor.matmul(
                                att[half][:, o:SC],
                                v_sb[j][:, 2 * d + half, :],
                                ex[:, SC * half + o : SC * (half + 1)],
                                start=(j == 0),
                                stop=(j == nt - 1),
                            )

                    prev = None
                    for j in range(nt):
                        m = j - 4 * sc
                        o = P * m if m > 0 else 0
                        sco = psc.tile([P, 2 * SC], F32, tag="sc", name="sco")
                        for half in range(2):
                            r = D * half
                            nc.tensor.matmul(
                                sco[:, SC * half + o : SC * (half + 1)],
                                kt[d][j // 4][r : r + D, P * (j % 4) : P * (j % 4 + 1)],
                                qt[d][sc][r : r + D, o:SC],
                                start=True,
                                stop=True,
                                tile_position=(r, 0),
                            )
                        ex = expp.tile([P, 2 * SC], BF16, tag="ex", name="ex")
                        if o > 0:
                            for half in range(2):
                                nc.scalar.activation(
                                    ex[:, SC * half + o : SC * (half + 1)],
                                    sco[:, SC * half + o : SC * (half + 1)],
                                    mybir.ActivationFunctionType.Exp,
                                    scale=SCALE,
                                )
                        else:
                            nc.scalar.activation(
                                ex[:],
                                sco[:],
                                mybir.ActivationFunctionType.Exp,
                                scale=SCALE,
                            )
                        if m >= 0:  # diagonal tile: mask the wedge
                            for half in range(2):
                                c0 = SC * half + P * m
                                nc.vector.tensor_mul(
                                    ex[:, c0 : c0 + P], ex[:, c0 : c0 + P], mask_sb[:]
                                )
                        if prev is not None:
                            emit_av(*prev)
                        prev = (j, o, ex)
                        it += 1
                        want = (it * len(fills)) // n_iter
                        while popped < want:
                            fills[popped]()
                            popped += 1
                    emit_av(*prev)
                    # stage raw AV + denominators, release att psum
                    attn_t = attnp.tile([P, SC], BF16, tag="at", name="attn_t")
                    for half in range(2):
                        nc.vector.tensor_copy(
                            attn_t[D * half : D * (half + 1), :], att[half][0:D, :]
                        )
                        dnrow = dnp.tile([1, SC], BF16, tag="dn", name="dnrow")
                        nc.vector.tensor_copy(dnrow[:], att[half][D : D + 1, :])
                        nc.sync.dma_start(
                            dn_loc[2 * d + half : 2 * d + half + 1, :], dnrow[:]
                        )
                    at.append(attn_t)
                while popped < len(fills):
                    fills[popped]()
                    popped += 1
                return at, dn_loc

            # ---- schedule ----
            for d in range(ND):
                emit_k(d, 0)
            for d in range(ND):
                emit_q(d, 0)
            for t in range(4):
                emit_v(t)

            chunks = {}
            for sc in range(NSC):
                fills = []
                if sc + 1 < NSC:
                    # Q first: chunk sc+1 needs it immediately; K/V tiles of
                    # key-chunk sc+1 aren't touched until late in sc+1
                    for d in range(ND):
                        fills.append(lambda d=d, qc=sc + 1: emit_q(d, qc))
                    for d in range(ND):
                        fills.append(lambda d=d, kc=sc + 1: emit_k(d, kc))
                    for t in range(4 * (sc + 1), 4 * (sc + 2)):
                        fills.append(lambda t=t: emit_v(t))
                # out-projections shifted late: the tail chunks are the
                # ACT-bound ones that need PE fill work
                if sc == 2:
                    fills.extend(out_projection_fills(0, *chunks[0]))
                if sc == 3:
                    fills.extend(out_projection_fills(1, *chunks[1]))
                    fills.extend(out_projection_fills(2, *chunks[2]))
                chunks[sc] = attention_chunk(sc, fills)
            for f in out_projection_fills(
                NSC - 1, *chunks[NSC - 1], sce=True, qsplit=True
            ):
                f()

    nc.compile()
    return nc


def _get_runner():
    """Build (once) and return a callable in_maps -> list of out_maps."""
    if "runner" in _CACHE:
        return _CACHE["runner"]

    nc = _build_nc()
    _CACHE["nc"] = nc

    import jax
    from jax.sharding import Mesh, PartitionSpec
    from jax.experimental.shard_map import shard_map
    from concourse import bass2jax
    from concourse.bass2jax import _bass_exec_p, partition_id_tensor

    bass2jax.install_neuronx_cc_hook()

    in_names, out_names, out_avals, zero_shapes = [], [], [], []
    partition_name = nc.partition_id_tensor.name if nc.partition_id_tensor else None
    for alloc in nc.m.functions[0].allocations:
        if not isinstance(alloc, mybir.MemoryLocationSet):
            continue
        name = alloc.memorylocations[0].name
        if alloc.kind == "ExternalInput":
            if name != partition_name:
                in_names.append(name)
        elif alloc.kind == "ExternalOutput":
            out_names.append(name)
            shape = tuple(alloc.tensor_shape)
            dtype = mybir.dt.np(alloc.dtype)
            out_avals.append(jax.core.ShapedArray(shape, dtype))
            zero_shapes.append((shape, dtype))
    n_params = len(in_names)
    all_in_names = list(in_names) + list(out_names)
    if partition_name is not None:
        all_in_names.append(partition_name)

    def _body(*args):
        operands = list(args)
        if partition_name is not None:
            operands.append(partition_id_tensor())
        outs = _bass_exec_p.bind(
            *operands,
            out_avals=tuple(out_avals),
            in_names=tuple(all_in_names),
            out_names=tuple(out_names),
            lowering_input_output_aliases=(),
            sim_require_finite=True,
            sim_require_nnan=True,
            nc=nc,
        )
        return tuple(outs)

    devices = jax.devices()[:NCORES]
    mesh = Mesh(np.asarray(devices), ("core",))
    n_outs = len(out_names)
    sharded = jax.jit(
        shard_map(
            _body,
            mesh=mesh,
            in_specs=(PartitionSpec("core"),) * (n_params + n_outs),
            out_specs=(PartitionSpec("core"),) * n_outs,
            check_rep=False,
        ),
        donate_argnums=tuple(range(n_params, n_params + n_outs)),
        keep_unused=True,
    )

    def runner(in_maps):
        per_core = [[np.asarray(m[name]) for name in in_names] for m in in_maps]
        concat_in = [
            np.concatenate([per_core[c][i] for c in range(NCORES)], axis=0)
            for i in range(n_params)
        ]
        concat_zeros = [
            np.zeros((NCORES * s[0], *s[1:]), d) for (s, d) in zero_shapes
        ]
        out_arrs = sharded(*concat_in, *concat_zeros)
        return [
            {
                name: np.asarray(out_arrs[i]).reshape(NCORES, *out_avals[i].shape)[c]
                for i, name in enumerate(out_names)
            }
            for c in range(NCORES)
        ]

    _CACHE["runner"] = runner
    _CACHE["sharded"] = sharded
    _CACHE["mesh"] = mesh
    _CACHE["meta"] = (in_names, out_names, zero_shapes)
    return runner


def timing_setup(in_maps):
    """Device-resident timing: returns (make_zeros, call).

    `call(make_zeros())` runs one on-device execution with inputs already
    resident (zeros are donated output buffers, created outside the timer).
    """
    _get_runner()
    import jax
    from jax.sharding import NamedSharding, PartitionSpec

    in_names, out_names, zero_shapes = _CACHE["meta"]
    sharding = NamedSharding(_CACHE["mesh"], PartitionSpec("core"))
    per_core = [[np.asarray(m[name]) for name in in_names] for m in in_maps]
    dev_in = [
        jax.device_put(
            np.concatenate([per_core[c][i] for c in range(NCORES)], axis=0), sharding
        )
        for i in range(len(in_names))
    ]
    jax.block_until_ready(dev_in)

    def make_zeros():
        zs = [
            jax.device_put(np.zeros((NCORES * s[0], *s[1:]), d), sharding)
            for (s, d) in zero_shapes
        ]
        jax.block_until_ready(zs)
        return zs

    def call(zs):
        out = _CACHE["sharded"](*dev_in, *zs)
        jax.block_until_ready(out)
        return out

    return make_zeros, call


def make_in_maps(x, Wq, Wk, Wv, Wo, bo):
    """Host-side sharding: slice/transpose/cast full inputs into per-core maps."""
    x = np.asarray(x, dtype=np.float32)
    Wq = np.asarray(Wq, dtype=np.float32)
    Wk = np.asarray(Wk, dtype=np.float32)
    Wv = np.asarray(Wv, dtype=np.float32)
    Wo = np.asarray(Wo, dtype=np.float32)
    bo = np.asarray(bo, dtype=np.float32)
    bf = ml_dtypes.bfloat16

    mask = np.triu(np.ones((P, P), dtype=bf))  # keep t <= s
    sel = np.zeros((HL, ND * P), dtype=bf)
    for d in range(ND):
        sel[2 * d, P * d : P * d + D] = 1
        sel[2 * d + 1, P * d + D : P * (d + 1)] = 1
    bo2 = np.ascontiguousarray((0.5 * bo).reshape(ET, P).T)  # [P, ET]
    WoT = np.ascontiguousarray(Wo.T)  # [dg_full, e]
    in_maps = []
    for c in range(NCORES):
        b, g = c // 2, c % 2
        xT = np.ascontiguousarray(x[b].T).astype(bf)  # [E, S]
        wq = np.ascontiguousarray(
            Wq[HL * g : HL * (g + 1)].transpose(1, 0, 2).reshape(E, DG)
        ).astype(bf)
        wk = np.ascontiguousarray(
            Wk[HL * g : HL * (g + 1)].transpose(1, 0, 2).reshape(E, DG)
        ).astype(bf)
        wv = np.ascontiguousarray(
            Wv[HL * g : HL * (g + 1)].transpose(1, 0, 2).reshape(E, DG)
        ).astype(bf)
        wo2 = np.ascontiguousarray(WoT[DG * g : DG * (g + 1), :]).astype(bf)
        in_maps.append(
            {
                "xT": xT,
                "wq": wq,
                "wk": wk,
                "wv": wv,
                "wo2": wo2,
                "bo2": bo2,
                "mask": mask,
                "sel8": sel,
            }
        )
    return in_maps


def assemble_output(results):
    """Gather per-core outT [EH, S] slices into the full [B, S, E] output."""
    out = np.empty((B, S, E), dtype=np.float32)
    for c in range(NCORES):
        b, g = c // 2, c % 2
        o = results[c]["outT"]  # [NSC, EH, SC]
        out[b, :, EH * g : EH * (g + 1)] = (
            o.transpose(0, 2, 1).reshape(S, EH).astype(np.float32)
        )
    return out


def kernel(x, Wq, Wk, Wv, Wo, bo):
    runner = _get_runner()
    in_maps = make_in_maps(x, Wq, Wk, Wv, Wo, bo)
    results = runner(in_maps)
    return assemble_output(results)


# revision 47
# speedup vs baseline: 1.1380x; 1.0074x over previous
"""Causal multi-head attention on 8 Trainium2 NeuronCores.

Problem: x[B=4,S=2048,E=1024], Wq/Wk/Wv[H=16,E,D=64], Wo[E,E], bo[E].
  out = softmax_causal(q k^T / sqrt(D)) v, heads concat, @ Wo.T + bo

Sharding (tensor parallel over heads, data parallel over batch):
  core c -> (batch b = c//2, head-group g = c%2 of 8 heads).
  Each core: QKV projections + attention for its 8 heads of its batch,
  normalization, and a FULL-width output-projection partial from its own
  heads (Wo split on its input axis); a pairwise ReduceScatter(add) over
  (2b, 2b+1) sums the partials and leaves each core its 512 output
  columns, written straight into the kernel output.  No peer data is ever
  needed by compute -> no collective on the critical path except the
  final add.

Kernel internals (per core), v3:
  - All SBUF data bf16 (psum f32).  Activations transposed: xT[E,S],
    QT/KT[dg,S], scoresT[t,q]; softmax denominator from a ones-column
    appended to V; probabilities feed the AV matmul moving operand.
  - Scores for a head PAIR (row-tiled 64-contraction matmuls at PE row
    groups 0/64) share one [128, 1024] psum tile; ONE exp activation
    covers both heads.
  - Causality at 128 granularity: suffix-restricted scores/exp/AV on
    diagonal tiles + a [128,128] upper-tri wedge multiply.
  - 1/denom via reciprocal_approx_fast on a per-chunk [8,512] stack
    (gathered by tiny SBUF->SBUF DMAs); broadcast across each head's 64
    rows by a small selector matmul; bf16 in-place normalize.
  - Phase-1 (QKV) and out-projection work interleaved into the attention
    emission so TensorE always has dense work while ScalarE exp runs
    (HAM stays at 2.4 GHz).  x is loaded in [128,512] column slices so
    the prologue is never DMA-starved.
"""

import os
import sys

for _p in ("/opt/trn_rl_repo", "/root/.axon_site/_ro/trn_rl_repo"):
    if os.path.isdir(_p) and _p not in sys.path:
        sys.path.append(_p)

import numpy as np
import ml_dtypes

import concourse.bass as bass
import concourse.mybir as mybir
import concourse.tile as tile
from concourse import bacc

B, S, E, H, D = 4, 2048, 1024, 16, 64
NCORES = 8
G = 2  # head groups
HL = H // G  # heads per core = 8
DG = HL * D  # local head dim = 512
EH = E // G  # final output columns per core = 512
P = 128
SC = 512  # sequence chunk
NSC = S // SC  # 4
NT = S // P  # 16 key tiles
ET = E // P  # 8 embedding tiles
ND = DG // P  # 4 head-pairs per core
SCALE = 1.0 / np.sqrt(D)

F32 = mybir.dt.float32
BF16 = mybir.dt.bfloat16

_CACHE = {}


def _build_nc():
    nc = bacc.Bacc("TRN2", target_bir_lowering=False, debug=False, num_devices=NCORES)

    xT = nc.dram_tensor("xT", [E, S], BF16, kind="ExternalInput")
    wq = nc.dram_tensor("wq", [E, DG], BF16, kind="ExternalInput")
    wk = nc.dram_tensor("wk", [E, DG], BF16, kind="ExternalInput")
    wv = nc.dram_tensor("wv", [E, DG], BF16, kind="ExternalInput")
    wo2 = nc.dram_tensor("wo2", [DG, E], BF16, kind="ExternalInput")
    bo2 = nc.dram_tensor("bo2", [P, ET], F32, kind="ExternalInput")
    mask = nc.dram_tensor("mask", [P, P], BF16, kind="ExternalInput")
    sel8 = nc.dram_tensor("sel8", [HL, ND * P], BF16, kind="ExternalInput")
    outT = nc.dram_tensor("outT", [NSC, EH, SC], BF16, kind="ExternalOutput")

    with tile.TileContext(nc) as tc:
        with (
            tc.tile_pool(name="persist", bufs=1) as persist,
            tc.tile_pool(name="expp", bufs=6) as expp,
            tc.tile_pool(name="attnp", bufs=16) as attnp,
            tc.tile_pool(name="dnp", bufs=5) as dnp,
            tc.tile_pool(name="dinvp", bufs=3) as dinvp,
            tc.tile_pool(name="workp", bufs=6) as workp,
            tc.tile_pool(name="outp", bufs=4) as outp,
            tc.tile_pool(name="psc", bufs=2, space="PSUM") as psc,
            tc.tile_pool(name="patt", bufs=2, space="PSUM") as patt,
            tc.tile_pool(name="pmisc", bufs=2, space="PSUM") as pmisc,
            tc.tile_pool(name="dram", bufs=1, space="DRAM") as dram,
        ):
            # ---- persistent tiles ----
            xs = [
                [persist.tile([P, SC], BF16, name=f"x{e}_{c}") for c in range(NSC)]
                for e in range(ET)
            ]
            wq_sb = persist.tile([P, ET, DG], BF16, name="wq")
            wk_sb = persist.tile([P, ET, DG], BF16, name="wk")
            wv_sb = persist.tile([P, ET, DG], BF16, name="wv")
            wo_sb = persist.tile([P, ND, E], BF16, name="wo")
            bo_sb = persist.tile([P, ET], F32, name="bo")
            mask_sb = persist.tile([P, P], BF16, name="mask")
            sel_sb = persist.tile([HL, ND * P], BF16, name="sel")
            kt = [
                [persist.tile([P, SC], BF16, name=f"kt{d}_{kc}") for kc in range(NSC)]
                for d in range(ND)
            ]
            qt = [
                [persist.tile([P, SC], BF16, name=f"qt{d}_{sc}") for sc in range(NSC)]
                for d in range(ND)
            ]
            v_sb = [persist.tile([P, HL, D + 1], BF16, name=f"v{t}") for t in range(NT)]

            cc_in = dram.tile([NSC, E, SC], BF16)
            cc_rs = dram.tile([NSC, 2, EH // 2, SC], BF16)
            # last chunk: query-halved layout so its two ReduceScatters
            # pipeline with the projection instead of fully serializing
            cc_q = dram.tile([2, E, SC // 2], BF16)
            cc_rs_q = dram.tile([2, EH, SC // 2], BF16)

            # ---- input DMAs: per-e weight blocks interleaved with x slices
            # so K(0)'s e-th matmul can start as soon as its operands land
            for e in range(ET):
                nc.sync.dma_start(
                    wk_sb[:, e, :], wk[P * e : P * (e + 1), :]
                )
                nc.sync.dma_start(xs[e][0][:], xT[P * e : P * (e + 1), 0:SC])
            for e in range(ET):
                nc.sync.dma_start(wq_sb[:, e, :], wq[P * e : P * (e + 1), :])
            for e in range(ET):
                nc.sync.dma_start(wv_sb[:, e, :], wv[P * e : P * (e + 1), :])
            for c in range(1, NSC):
                for e in range(ET):
                    nc.sync.dma_start(
                        xs[e][c][:], xT[P * e : P * (e + 1), SC * c : SC * (c + 1)]
                    )
            nc.sync.dma_start(wo_sb[:], wo2.rearrange("(ko p) m -> p ko m", p=P))
            nc.sync.dma_start(bo_sb[:], bo2[:])
            nc.sync.dma_start(mask_sb[:], mask[:])
            nc.sync.dma_start(sel_sb[:], sel8[:])
            for t in range(NT):
                nc.vector.memset(v_sb[t][:, :, D], 1.0)

            # ---- phase-1 emitters ----
            def emit_k(d, kc, w_sb=wk_sb, dst=kt):
                acc = pmisc.tile([P, SC], F32, tag="m", name="acc")
                for e in range(ET):
                    nc.tensor.matmul(
                        acc[:],
                        w_sb[:, e, P * d : P * (d + 1)],
                        xs[e][kc][:],
                        start=(e == 0),
                        stop=(e == ET - 1),
                    )
                nc.vector.tensor_copy(dst[d][kc][:], acc[:])

            def emit_q(d, sc):
                emit_k(d, sc, w_sb=wq_sb, dst=qt)

            def emit_v(t):
                acc = pmisc.tile([P, DG], F32, tag="m", name="accv")
                for e in range(ET):
                    nc.tensor.matmul(
                        acc[:],
                        xs[e][t // 4][:, P * (t % 4) : P * (t % 4 + 1)],
                        wv_sb[:, e, :],
                        start=(e == 0),
                        stop=(e == ET - 1),
                    )
                nc.vector.tensor_copy(
                    v_sb[t][:, :, 0:D], acc[:].rearrange("p (h d) -> p h d", d=D)
                )

            # ---- normalization (deferred fills; dn_loc complete by then) ----
            def op_dinv(dinv_b, dn_loc, sce=False):
                # sce: route casts via ScalarE (idle in the epilogue)
                cp = nc.scalar.copy if sce else nc.vector.tensor_copy
                dn_f = dinvp.tile([HL, SC], F32, tag="dnf", name="dn_f")
                dn_i = dinvp.tile([HL, SC], F32, tag="dni", name="dn_i")
                cp(dn_f[:], dn_loc[:])
                nc.vector.reciprocal_approx_fast(dn_i[:], dn_f[:])
                cp(dinv_b[:], dn_i[:])

            def op_norm(d, at, dinv_b, sce=False):
                bc = pmisc.tile([P, SC], F32, tag="m", name="bc")
                nc.tensor.matmul(
                    bc[:], sel_sb[:, P * d : P * (d + 1)], dinv_b[:],
                    start=True, stop=True,
                )
                bc_sb = workp.tile([P, SC], BF16, tag="bc")
                (nc.scalar.copy if sce else nc.vector.tensor_copy)(bc_sb[:], bc[:])
                nc.vector.tensor_mul(at[d][:], at[d][:], bc_sb[:])

            # cc_in row order interleaves the two ranks' e-quarters so each
            # half [0:512] / [512:1024] ReduceScatters to the right slices:
            # rows = [e 0:256 | e 512:768 | e 256:512 | e 768:1024]
            CPERM = {0: 0, 1: 1, 2: 4, 3: 5, 4: 2, 5: 3, 6: 6, 7: 7}

            def op_proj(sc, et, at):
                acc = pmisc.tile([P, SC], F32, tag="m", name="acco")
                for d in range(ND):
                    nc.tensor.matmul(
                        acc[:],
                        wo_sb[:, d, P * et : P * (et + 1)],
                        at[d][:],
                        start=(d == 0),
                        stop=(d == ND - 1),
                    )
                stage = outp.tile([P, SC], BF16, tag="ot")
                nc.vector.tensor_scalar_add(stage[:], acc[:], bo_sb[:, et : et + 1])
                r = P * CPERM[et]
                nc.sync.dma_start(cc_in[sc, r : r + P, :], stage[:])

            def op_rs(sc, h):
                nc.gpsimd.collective_compute(
                    "ReduceScatter",
                    mybir.AluOpType.add,
                    replica_groups=[[0, 1], [2, 3], [4, 5], [6, 7]],
                    ins=[cc_in[sc, (E // 2) * h : (E // 2) * (h + 1), :].opt()],
                    outs=[cc_rs[sc, h].opt()],
                )
                nc.sync.dma_start(
                    outT[sc, (EH // 2) * h : (EH // 2) * (h + 1), :], cc_rs[sc, h]
                )

            def op_proj_q(et, qh, at):
                q0 = (SC // 2) * qh
                acc = pmisc.tile([P, SC // 2], F32, tag="m", name="accq")
                for d in range(ND):
                    nc.tensor.matmul(
                        acc[:],
                        wo_sb[:, d, P * et : P * (et + 1)],
                        at[d][:, q0 : q0 + SC // 2],
                        start=(d == 0),
                        stop=(d == ND - 1),
                    )
                stage = outp.tile([P, SC // 2], BF16, tag="otq")
                nc.vector.tensor_scalar_add(stage[:], acc[:], bo_sb[:, et : et + 1])
                # single 1024-row ReduceScatter: rank r takes rows [512r:512r+512],
                # so et blocks stay in identity order (no CPERM here)
                nc.sync.dma_start(cc_q[qh, P * et : P * (et + 1), :], stage[:])

            def op_rs_q(sc, qh):
                nc.gpsimd.collective_compute(
                    "ReduceScatter",
                    mybir.AluOpType.add,
                    replica_groups=[[0, 1], [2, 3], [4, 5], [6, 7]],
                    ins=[cc_q[qh].opt()],
                    outs=[cc_rs_q[qh].opt()],
                )
                q0 = (SC // 2) * qh
                nc.sync.dma_start(outT[sc, :, q0 : q0 + SC // 2], cc_rs_q[qh])

            def out_projection_fills(sc, at, dn_loc, sce=False, qsplit=False):
                dinv_b = dinvp.tile([HL, SC], BF16, tag="dnb", name="dinv_b")
                fills = [lambda: op_dinv(dinv_b, dn_loc, sce)]
                for d in range(ND):
                    fills.append(lambda d=d: op_norm(d, at, dinv_b, sce))
                if qsplit:
                    for qh in range(2):
                        for et in range(ET):
                            fills.append(lambda et=et, qh=qh: op_proj_q(et, qh, at))
                        fills.append(lambda qh=qh: op_rs_q(sc, qh))
                else:
                    for h in range(2):
                        for et in ([0, 1, 4, 5] if h == 0 else [2, 3, 6, 7]):
                            fills.append(lambda et=et: op_proj(sc, et, at))
                        fills.append(lambda h=h: op_rs(sc, h))
                return fills

            # ---- attention ----
            def attention_chunk(sc, fills):
                nt = 4 * (sc + 1)
                n_iter = ND * nt
                it = 0
                popped = 0
                at = []  # per-d AV tiles [128, 512] (bf16), normalized by fills
                dn_loc = dnp.tile([HL, SC], BF16, tag="dnl", name="dn_loc")
                for d in range(ND):
                    att = [
                        patt.tile([D + 1, SC], F32, tag="att", name=f"att{h}")
                        for h in range(2)
                    ]

                    def emit_av(j, o, ex, d=d, att=att, nt=nt):
                        for half in range(2):
                            nc.tensor.matmul(
                                att[half][:, o:SC],
                                v_sb[j][:, 2 * d + half, :],
                                ex[:, SC * half + o : SC * (half + 1)],
                                start=(j == 0),
                                stop=(j == nt - 1),
                            )

                    prev = None
                    for j in range(nt):
                        m = j - 4 * sc
                        o = P * m if m > 0 else 0
                        sco = psc.tile([P, 2 * SC], F32, tag="sc", name="sco")
                        for half in range(2):
                            r = D * half
                            nc.tensor.matmul(
                                sco[:, SC * half + o : SC * (half + 1)],
                                kt[d][j // 4][r : r + D, P * (j % 4) : P * (j % 4 + 1)],
                                qt[d][sc][r : r + D, o:SC],
                                start=True,
                                stop=True,
                                tile_position=(r, 0),
                            )
                        ex = expp.tile([P, 2 * SC], BF16, tag="ex", name="ex")
                        if o > 0:
                            for half in range(2):
                                nc.scalar.activation(
                                    ex[:, SC * half + o : SC * (half + 1)],
                                    sco[:, SC * half + o : SC * (half + 1)],
                                    mybir.ActivationFunctionType.Exp,
                                    scale=SCALE,
                                )
                        else:
                            nc.scalar.activation(
                                ex[:],
                                sco[:],
                                mybir.ActivationFunctionType.Exp,
                                scale=SCALE,
                            )
                        if m >= 0:  # diagonal tile: mask the wedge
                            for half in range(2):
                                c0 = SC * half + P * m
                                nc.vector.tensor_mul(
                                    ex[:, c0 : c0 + P], ex[:, c0 : c0 + P], mask_sb[:]
                                )
                        if prev is not None:
                            emit_av(*prev)
                        prev = (j, o, ex)
                        it += 1
                        want = (it * len(fills)) // n_iter
                        while popped < want:
                            fills[popped]()
                            popped += 1
                    emit_av(*prev)
                    # stage raw AV + denominators, release att psum
                    attn_t = attnp.tile([P, SC], BF16, tag="at", name="attn_t")
                    for half in range(2):
                        nc.vector.tensor_copy(
                            attn_t[D * half : D * (half + 1), :], att[half][0:D, :]
                        )
                        dnrow = dnp.tile([1, SC], BF16, tag="dn", name="dnrow")
                        nc.vector.tensor_copy(dnrow[:], att[half][D : D + 1, :])
                        nc.sync.dma_start(
                            dn_loc[2 * d + half : 2 * d + half + 1, :], dnrow[:]
                        )
                    at.append(attn_t)
                while popped < len(fills):
                    fills[popped]()
                    popped += 1
                return at, dn_loc

            # ---- schedule ----
            # minimal prologue: only what chunk-0 d0's attention needs
            emit_k(0, 0)
            emit_q(0, 0)
            for t in range(4):
                emit_v(t)

            chunks = {}
            for sc in range(NSC):
                fills = []
                if sc == 0:
                    # rest of the prologue: K/Q for head-pairs 1-3 of chunk 0,
                    # interleaved so chunk-0 d0's attention starts immediately
                    for d in range(1, ND):
                        fills.append(lambda d=d: emit_k(d, 0))
                        fills.append(lambda d=d: emit_q(d, 0))
                if sc + 1 < NSC:
                    # Q first: chunk sc+1 needs it immediately; K/V tiles of
                    # key-chunk sc+1 aren't touched until late in sc+1
                    for d in range(ND):
                        fills.append(lambda d=d, qc=sc + 1: emit_q(d, qc))
                    for d in range(ND):
                        fills.append(lambda d=d, kc=sc + 1: emit_k(d, kc))
                    for t in range(4 * (sc + 1), 4 * (sc + 2)):
                        fills.append(lambda t=t: emit_v(t))
                # out-projections shifted late: the tail chunks are the
                # ACT-bound ones that need PE fill work
                if sc == 2:
                    fills.extend(out_projection_fills(0, *chunks[0]))
                if sc == 3:
                    fills.extend(out_projection_fills(1, *chunks[1]))
                    fills.extend(out_projection_fills(2, *chunks[2]))
                chunks[sc] = attention_chunk(sc, fills)
            for f in out_projection_fills(
                NSC - 1, *chunks[NSC - 1], sce=True, qsplit=True
            ):
                f()

    nc.compile()
    return nc


def _get_runner():
    """Build (once) and return a callable in_maps -> list of out_maps."""
    if "runner" in _CACHE:
        return _CACHE["runner"]

    nc = _build_nc()
    _CACHE["nc"] = nc

    import jax
    from jax.sharding import Mesh, PartitionSpec
    from jax.experimental.shard_map import shard_map
    from concourse import bass2jax
    from concourse.bass2jax import _bass_exec_p, partition_id_tensor

    bass2jax.install_neuronx_cc_hook()

    in_names, out_names, out_avals, zero_shapes = [], [], [], []
    partition_name = nc.partition_id_tensor.name if nc.partition_id_tensor else None
    for alloc in nc.m.functions[0].allocations:
        if not isinstance(alloc, mybir.MemoryLocationSet):
            continue
        name = alloc.memorylocations[0].name
        if alloc.kind == "ExternalInput":
            if name != partition_name:
                in_names.append(name)
        elif alloc.kind == "ExternalOutput":
            out_names.append(name)
            shape = tuple(alloc.tensor_shape)
            dtype = mybir.dt.np(alloc.dtype)
            out_avals.append(jax.core.ShapedArray(shape, dtype))
            zero_shapes.append((shape, dtype))
    n_params = len(in_names)
    all_in_names = list(in_names) + list(out_names)
    if partition_name is not None:
        all_in_names.append(partition_name)

    def _body(*args):
        operands = list(args)
        if partition_name is not None:
            operands.append(partition_id_tensor())
        outs = _bass_exec_p.bind(
            *operands,
            out_avals=tuple(out_avals),
            in_names=tuple(all_in_names),
            out_names=tuple(out_names),
            lowering_input_output_aliases=(),
            sim_require_finite=True,
            sim_require_nnan=True,
            nc=nc,
        )
        return tuple(outs)

    devices = jax.devices()[:NCORES]
    mesh = Mesh(np.asarray(devices), ("core",))
    n_outs = len(out_names)
    sharded = jax.jit(
        shard_map(
            _body,
            mesh=mesh,
            in_specs=(PartitionSpec("core"),) * (n_params + n_outs),
            out_specs=(PartitionSpec("core"),) * n_outs,
            check_rep=False,
        ),
        donate_argnums=tuple(range(n_params, n_params + n_outs)),
        keep_unused=True,
    )

    def runner(in_maps):
        per_core = [[np.asarray(m[name]) for name in in_names] for m in in_maps]
        concat_in = [
            np.concatenate([per_core[c][i] for c in range(NCORES)], axis=0)
            for i in range(n_params)
        ]
        concat_zeros = [
            np.zeros((NCORES * s[0], *s[1:]), d) for (s, d) in zero_shapes
        ]
        out_arrs = sharded(*concat_in, *concat_zeros)
        return [
            {
                name: np.asarray(out_arrs[i]).reshape(NCORES, *out_avals[i].shape)[c]
                for i, name in enumerate(out_names)
            }
            for c in range(NCORES)
        ]

    _CACHE["runner"] = runner
    _CACHE["sharded"] = sharded
    _CACHE["mesh"] = mesh
    _CACHE["meta"] = (in_names, out_names, zero_shapes)
    return runner


def timing_setup(in_maps):
    """Device-resident timing: returns (make_zeros, call).

    `call(make_zeros())` runs one on-device execution with inputs already
    resident (zeros are donated output buffers, created outside the timer).
    """
    _get_runner()
    import jax
    from jax.sharding import NamedSharding, PartitionSpec

    in_names, out_names, zero_shapes = _CACHE["meta"]
    sharding = NamedSharding(_CACHE["mesh"], PartitionSpec("core"))
    per_core = [[np.asarray(m[name]) for name in in_names] for m in in_maps]
    dev_in = [
        jax.device_put(
            np.concatenate([per_core[c][i] for c in range(NCORES)], axis=0), sharding
        )
        for i in range(len(in_names))
    ]
    jax.block_until_ready(dev_in)

    def make_zeros():
        zs = [
            jax.device_put(np.zeros((NCORES * s[0], *s[1:]), d), sharding)
            for (s, d) in zero_shapes
        ]
        jax.block_until_ready(zs)
        return zs

    def call(zs):
        out = _CACHE["sharded"](*dev_in, *zs)
        jax.block_until_ready(out)
        return out

    return make_zeros, call


def make_in_maps(x, Wq, Wk, Wv, Wo, bo):
    """Host-side sharding: slice/transpose/cast full inputs into per-core maps."""
    x = np.asarray(x, dtype=np.float32)
    Wq = np.asarray(Wq, dtype=np.float32)
    Wk = np.asarray(Wk, dtype=np.float32)
    Wv = np.asarray(Wv, dtype=np.float32)
    Wo = np.asarray(Wo, dtype=np.float32)
    bo = np.asarray(bo, dtype=np.float32)
    bf = ml_dtypes.bfloat16

    mask = np.triu(np.ones((P, P), dtype=bf))  # keep t <= s
    sel = np.zeros((HL, ND * P), dtype=bf)
    for d in range(ND):
        sel[2 * d, P * d : P * d + D] = 1
        sel[2 * d + 1, P * d + D : P * (d + 1)] = 1
    bo2 = np.ascontiguousarray((0.5 * bo).reshape(ET, P).T)  # [P, ET]
    WoT = np.ascontiguousarray(Wo.T)  # [dg_full, e]
    in_maps = []
    for c in range(NCORES):
        b, g = c // 2, c % 2
        xT = np.ascontiguousarray(x[b].T).astype(bf)  # [E, S]
        wq = np.ascontiguousarray(
            Wq[HL * g : HL * (g + 1)].transpose(1, 0, 2).reshape(E, DG)
        ).astype(bf)
        wk = np.ascontiguousarray(
            Wk[HL * g : HL * (g + 1)].transpose(1, 0, 2).reshape(E, DG)
        ).astype(bf)
        wv = np.ascontiguousarray(
            Wv[HL * g : HL * (g + 1)].transpose(1, 0, 2).reshape(E, DG)
        ).astype(bf)
        wo2 = np.ascontiguousarray(WoT[DG * g : DG * (g + 1), :]).astype(bf)
        in_maps.append(
            {
                "xT": xT,
                "wq": wq,
                "wk": wk,
                "wv": wv,
                "wo2": wo2,
                "bo2": bo2,
                "mask": mask,
                "sel8": sel,
            }
        )
    return in_maps


def assemble_output(results):
    """Gather per-core outT [EH, S] slices into the full [B, S, E] output."""
    out = np.empty((B, S, E), dtype=np.float32)
    for c in range(NCORES):
        b, g = c // 2, c % 2
        o = results[c]["outT"]  # [NSC, EH, SC]
        out[b, :, EH * g : EH * (g + 1)] = (
            o.transpose(0, 2, 1).reshape(S, EH).astype(np.float32)
        )
    return out


def kernel(x, Wq, Wk, Wv, Wo, bo):
    runner = _get_runner()
    in_maps = make_in_maps(x, Wq, Wk, Wv, Wo, bo)
    results = runner(in_maps)
    return assemble_output(results)
